# revision 1
# baseline (speedup 1.0000x reference)
"""Trainium2 Bass kernel for naive causal MHA (dense transformer block).

Problem: x[2, 2048, 1024], per-head QKV (16 heads, head_dim 64), causal
softmax attention, concat heads, output projection.

Sharding (8 NeuronCores, tensor-parallel over heads):
  - core c computes QKV + attention for heads {2c, 2c+1} over both batches,
    entirely in a transposed layout: scores are built as [keys, queries] so
    the softmax denominator comes from an extra ones-column in V and the
    attention output lands directly in the [head_dim, seq] layout the output
    projection needs as its stationary operand. No on-device transposes.
  - an 8-way AllToAll reshards y from head-split to row-split,
  - each core computes a disjoint 512-row slice of y @ Wout + bout.
The host only slices/transposes inputs and concatenates the 8 row-slices.

All matmuls run in float32r (single-pass FP22 on the PE array).
"""

import contextlib
import ctypes
import sys
import types

import numpy as np

import concourse.bacc as bacc
import concourse.mybir as mybir
import concourse.tile as tile
from concourse.bass import ds

N_CORES = 8
B = 2
S = 2048
D = 1024
HD = 64
N_HEADS = 16

DT = mybir.dt.float32
DTR = mybir.dt.float32r

SC = 512          # seq chunk (moving-operand width)
N_SC = S // SC    # 4
N_DC = D // 128   # 8 contraction chunks
N_SB = S // 128   # 16 seq 128-blocks


def _f32r(ap):
    return ap.bitcast(DTR)


def _mask_np():
    """mask4[j] for the expT tile [t=128, q=512] whose t-block is the j-th
    diagonal block of the q-chunk: q-subblocks < j are zero, == j are
    upper-triangular (keep t <= q), > j are ones."""
    m = np.zeros((4, 128, SC), dtype=np.float32)
    tri = np.triu(np.ones((128, 128), dtype=np.float32))
    for j in range(4):
        m[j, :, j * 128 : (j + 1) * 128] = tri
        m[j, :, (j + 1) * 128 :] = 1.0
    return m


def _build_program(dbg=False):
    nc = bacc.Bacc(
        "TRN2", target_bir_lowering=False, debug=False, num_devices=N_CORES
    )

    xt_d = nc.dram_tensor("xt", [B, D, S], DT, kind="ExternalInput").ap()
    wq_d = nc.dram_tensor("wq", [D, 128], DT, kind="ExternalInput").ap()
    wk_d = nc.dram_tensor("wk", [D, 128], DT, kind="ExternalInput").ap()
    wv_d = nc.dram_tensor("wv", [D, 128], DT, kind="ExternalInput").ap()
    bq_d = nc.dram_tensor("bq", [128, 1], DT, kind="ExternalInput").ap()
    bk_d = nc.dram_tensor("bk", [128, 1], DT, kind="ExternalInput").ap()
    bv_d = nc.dram_tensor("bv", [1, 128], DT, kind="ExternalInput").ap()
    wout_d = nc.dram_tensor("wout", [D, D], DT, kind="ExternalInput").ap()
    bout_d = nc.dram_tensor("bout", [1, D], DT, kind="ExternalInput").ap()
    out_d = nc.dram_tensor("out", [512, D], DT, kind="ExternalOutput").ap()

    y_part = nc.dram_tensor("y_part", [8, 128, 512], DT)
    y_all = nc.dram_tensor("y_all", [8, 128, 512], DT)
    if dbg:
        dbg_qT = nc.dram_tensor("dbg_qT", [B, 128, S], DT, kind="ExternalOutput").ap()
        dbg_kT = nc.dram_tensor("dbg_kT", [B, 128, S], DT, kind="ExternalOutput").ap()
        dbg_v = nc.dram_tensor("dbg_v", [B, 128, N_SB * 2 * 65], DT, kind="ExternalOutput").ap()
        dbg_yp = nc.dram_tensor("dbg_yp", [8, 128, 512], DT, kind="ExternalOutput").ap()
        dbg_ya = nc.dram_tensor("dbg_ya", [8, 128, 512], DT, kind="ExternalOutput").ap()

    mask_d = nc.inline_tensor(_mask_np(), name="mask4")
    ones_d = nc.inline_tensor(
        np.ones((128, N_SB, 2, 1), dtype=np.float32), name="vones"
    )

    with tile.TileContext(nc) as tc, contextlib.ExitStack() as ctx:
        const = ctx.enter_context(tc.tile_pool(name="const", bufs=1))
        xt_pool = ctx.enter_context(tc.tile_pool(name="xt", bufs=9))
        qk_pool = ctx.enter_context(tc.tile_pool(name="qk", bufs=2))
        v_pool = ctx.enter_context(tc.tile_pool(name="vp", bufs=2))
        exp_pool = ctx.enter_context(tc.tile_pool(name="expp", bufs=6))
        zr_pool = ctx.enter_context(tc.tile_pool(name="zr", bufs=2))
        zb_pool = ctx.enter_context(tc.tile_pool(name="zb", bufs=3))
        yts_pool = ctx.enter_context(tc.tile_pool(name="yts", bufs=3))
        yg_pool = ctx.enter_context(tc.tile_pool(name="yg", bufs=1))
        outs_pool = ctx.enter_context(tc.tile_pool(name="outs", bufs=3))
        psum = ctx.enter_context(tc.tile_pool(name="psum", bufs=2, space="PSUM"))
        dram_pool = ctx.enter_context(tc.tile_pool(name="dram", bufs=4, space="DRAM"))

        # ---- constants into SBUF ----
        wq_sb = const.tile([128, N_DC, 128], DT)
        nc.sync.dma_start(out=_f32r(wq_sb), in_=_f32r(wq_d.rearrange("(c p) e -> p c e", p=128)))
        wk_sb = const.tile([128, N_DC, 128], DT)
        nc.sync.dma_start(out=_f32r(wk_sb), in_=_f32r(wk_d.rearrange("(c p) e -> p c e", p=128)))
        wv_sb = const.tile([128, N_DC, 128], DT)
        nc.sync.dma_start(out=_f32r(wv_sb), in_=_f32r(wv_d.rearrange("(c p) e -> p c e", p=128)))
        wout_sb = const.tile([128, N_DC, D], DT)
        nc.sync.dma_start(out=_f32r(wout_sb), in_=_f32r(wout_d.rearrange("(c p) e -> p c e", p=128)))
        bq_sb = const.tile([128, 1], DT)
        nc.sync.dma_start(out=bq_sb, in_=bq_d)
        bk_sb = const.tile([128, 1], DT)
        nc.sync.dma_start(out=bk_sb, in_=bk_d)
        bv_bc = const.tile([128, 128], DT)
        nc.sync.dma_start(out=bv_bc, in_=bv_d.to_broadcast([128, 128]))
        bout_bc = const.tile([128, D], DT)
        nc.sync.dma_start(out=bout_bc, in_=bout_d.to_broadcast([128, D]))
        mask_sb = const.tile([128, 4, SC], DT)
        nc.sync.dma_start(out=mask_sb, in_=mask_d.ap().transpose([1, 0, 2]))

        for b in range(B):
            # ---- QKV projection for batch b ----
            qT = qk_pool.tile([128, S], DT, tag="qT")
            kT = qk_pool.tile([128, S], DT, tag="kT")
            v_sb = v_pool.tile([128, N_SB, 2, 65], DT)
            nc.sync.dma_start(
                out=_f32r(v_sb[:, :, :, 64:65]), in_=_f32r(ones_d.ap())
            )
            for sc in range(N_SC):
                xts = []
                for dc in range(N_DC):
                    xt = xt_pool.tile([128, SC], DT)
                    nc.sync.dma_start(
                        out=_f32r(xt),
                        in_=_f32r(xt_d[b, ds(dc * 128, 128), ds(sc * SC, SC)]),
                    )
                    xts.append(xt)
                psq = psum.tile([128, SC], DT, tag="psq", bufs=1)
                for dc in range(N_DC):
                    nc.tensor.matmul(
                        psq, _f32r(wq_sb[:, dc, :]), _f32r(xts[dc]),
                        start=(dc == 0), stop=(dc == N_DC - 1),
                    )
                nc.vector.tensor_scalar_add(
                    out=_f32r(qT[:, ds(sc * SC, SC)]), in0=psq, scalar1=bq_sb
                )
                psk = psum.tile([128, SC], DT, tag="psk", bufs=1)
                for dc in range(N_DC):
                    nc.tensor.matmul(
                        psk, _f32r(wk_sb[:, dc, :]), _f32r(xts[dc]),
                        start=(dc == 0), stop=(dc == N_DC - 1),
                    )
                nc.vector.tensor_scalar_add(
                    out=_f32r(kT[:, ds(sc * SC, SC)]), in0=psk, scalar1=bk_sb
                )
                for j4 in range(4):
                    psv = psum.tile([128, 128], DT, tag="psv", bufs=1)
                    for dc in range(N_DC):
                        nc.tensor.matmul(
                            psv,
                            _f32r(xts[dc][:, ds(j4 * 128, 128)]),
                            _f32r(wv_sb[:, dc, :]),
                            start=(dc == 0), stop=(dc == N_DC - 1),
                        )
                    sb_i = sc * 4 + j4
                    nc.vector.tensor_add(
                        out=_f32r(v_sb[:, sb_i, :, 0:64]),
                        in0=psv.rearrange("p (h e) -> p h e", h=2),
                        in1=bv_bc.rearrange("p (h e) -> p h e", h=2),
                    )

            # ---- attention for batch b (2 heads) ----
            # copy head-1 rows down to base partition 0: all matmul operands
            # at base 0 (base-64 operand pairs misbehave on HW)
            qT1 = qk_pool.tile([64, S], DT, tag="qT1", bufs=1)
            nc.sync.dma_start(out=_f32r(qT1), in_=_f32r(qT[64:128, :]))
            kT1 = qk_pool.tile([64, S], DT, tag="kT1", bufs=1)
            nc.sync.dma_start(out=_f32r(kT1), in_=_f32r(kT[64:128, :]))
            for qc in range(N_SC):
                ntb = 4 * qc + 4
                # interleave both heads' score->exp->AV chains so the PE's
                # in-order AV matmuls hide the other head's exp latency
                psy0 = psum.tile([65, SC], DT, tag="psy", bufs=2)
                psy1 = psum.tile([65, SC], DT, tag="psy", bufs=2)
                psys = [psy0, psy1]
                for tb in range(ntb):
                    exs = []
                    for h in range(2):
                        qTh, kTh = (qT, kT) if h == 0 else (qT1, kT1)
                        pss = psum.tile([128, SC], DT, tag="pss", bufs=3)
                        nc.tensor.matmul(
                            pss,
                            _f32r(kTh[0:64, ds(tb * 128, 128)]),
                            _f32r(qTh[0:64, ds(qc * SC, SC)]),
                            start=True, stop=True,
                        )
                        ex = exp_pool.tile([128, SC], DT)
                        nc.scalar.activation(
                            out=_f32r(ex), in_=pss,
                            func=mybir.ActivationFunctionType.Exp,
                            scale=0.125,
                        )
                        j = tb - 4 * qc
                        if j >= 0:
                            nc.vector.tensor_mul(
                                out=_f32r(ex), in0=_f32r(ex), in1=mask_sb[:, j, :]
                            )
                        exs.append(ex)
                    for h in range(2):
                        nc.tensor.matmul(
                            psys[h], _f32r(v_sb[:, tb, h, :]), _f32r(exs[h]),
                            start=(tb == 0), stop=(tb == ntb - 1),
                        )
                for h in range(2):
                    hb = 64 * h
                    psy = psys[h]
                    # normalize: row 64 of psy is sum(exp)
                    zr = zr_pool.tile([65, SC], DT)
                    nc.vector.reciprocal(out=zr[64:65, :], in_=psy[64:65, :])
                    zd = dram_pool.tile([1, SC], DT)
                    nc.sync.dma_start(out=zd, in_=zr[64:65, :])
                    zb = zb_pool.tile([64, SC], DT)
                    nc.sync.dma_start(out=zb, in_=zd.to_broadcast([64, SC]))
                    yts = yts_pool.tile([64, SC], DT)
                    nc.vector.tensor_mul(out=yts, in0=psy[0:64, :], in1=zb)
                    nc.sync.dma_start(
                        out=y_part.ap()[b * 4 + qc, ds(hb, 64), :], in_=yts
                    )

            if dbg:
                nc.sync.dma_start(out=dbg_qT[b], in_=qT)
                nc.sync.dma_start(out=dbg_kT[b], in_=kT)
                nc.sync.dma_start(
                    out=dbg_v[b], in_=v_sb.rearrange("p a b c -> p (a b c)")
                )

        # ---- reshard: head-split -> row-split ----
        nc.gpsimd.collective_compute(
            "AllToAll",
            mybir.AluOpType.bypass,
            replica_groups=[list(range(N_CORES))],
            ins=[y_part.ap()],
            outs=[y_all.ap()],
        )

        if dbg:
            nc.sync.dma_start(out=dbg_yp, in_=y_part.ap())
            nc.sync.dma_start(out=dbg_ya, in_=y_all.ap())

        # ---- output projection for this core's 512 rows ----
        ygs = []
        for ec in range(8):
            yg = yg_pool.tile([128, 512], DT, tag=f"yg{ec}")
            nc.sync.dma_start(out=_f32r(yg), in_=_f32r(y_all.ap()[ec]))
            ygs.append(yg)
        for sb in range(4):
            for ch in range(2):
                pso = psum.tile([128, SC], DT, tag="pss", bufs=3)
                for ec in range(8):
                    nc.tensor.matmul(
                        pso,
                        _f32r(ygs[ec][:, ds(sb * 128, 128)]),
                        _f32r(wout_sb[:, ec, ds(ch * SC, SC)]),
                        start=(ec == 0), stop=(ec == 7),
                    )
                ot = outs_pool.tile([128, SC], DT)
                nc.vector.tensor_add(
                    out=ot, in0=pso, in1=bout_bc[:, ds(ch * SC, SC)]
                )
                nc.sync.dma_start(
                    out=out_d[ds(sb * 128, 128), ds(ch * SC, SC)], in_=ot
                )

    nc.compile()
    return nc


_NC_CACHE = None


def _get_program():
    global _NC_CACHE
    if _NC_CACHE is None:
        _NC_CACHE = _build_program()
    return _NC_CACHE


def make_in_maps(x, Wqkv, bqkv, Wout, bout):
    x = np.asarray(x, dtype=np.float32)
    Wqkv = np.asarray(Wqkv, dtype=np.float32)
    bqkv = np.asarray(bqkv, dtype=np.float32)
    Wout = np.asarray(Wout, dtype=np.float32)
    bout = np.asarray(bout, dtype=np.float32)

    xt = np.ascontiguousarray(x.transpose(0, 2, 1))  # [B, D, S]
    wout = np.ascontiguousarray(Wout)
    bout2 = np.ascontiguousarray(bout.reshape(1, D))

    in_maps = []
    for c in range(N_CORES):
        h0, h1 = 2 * c, 2 * c + 1
        wq = np.ascontiguousarray(
            np.concatenate([Wqkv[h0, :, 0:64], Wqkv[h1, :, 0:64]], axis=1)
        )
        wk = np.ascontiguousarray(
            np.concatenate([Wqkv[h0, :, 64:128], Wqkv[h1, :, 64:128]], axis=1)
        )
        wv = np.ascontiguousarray(
            np.concatenate([Wqkv[h0, :, 128:192], Wqkv[h1, :, 128:192]], axis=1)
        )
        bq = np.ascontiguousarray(
            np.concatenate([bqkv[h0, 0:64], bqkv[h1, 0:64]]).reshape(128, 1)
        )
        bk = np.ascontiguousarray(
            np.concatenate([bqkv[h0, 64:128], bqkv[h1, 64:128]]).reshape(128, 1)
        )
        bv = np.ascontiguousarray(
            np.concatenate([bqkv[h0, 128:192], bqkv[h1, 128:192]]).reshape(1, 128)
        )
        in_maps.append(
            {
                "xt": xt,
                "wq": wq,
                "wk": wk,
                "wv": wv,
                "bq": bq,
                "bk": bk,
                "bv": bv,
                "wout": wout,
                "bout": bout2,
            }
        )
    return in_maps


def assemble(results):
    full = np.empty((N_CORES * 512, D), dtype=np.float32)
    for c in range(N_CORES):
        full[512 * c : 512 * (c + 1)] = results[c]["out"]
    return full.reshape(B, S, D)


def _install_ntff_hook():
    """The agent image's antenv lacks axon_hooks; provide it so
    run_bass_kernel_spmd(trace=True) can NTFF-profile via libaxon."""
    if "antenv.axon_hooks" in sys.modules:
        return
    so_path = "/opt/axon/libaxon_pjrt.so"
    try:
        lib = ctypes.CDLL(so_path)
        lib.axon_start_nrt_profile.argtypes = [
            ctypes.POINTER(ctypes.c_int64),
            ctypes.c_size_t,
        ]
        lib.axon_start_nrt_profile.restype = ctypes.c_int64
        lib.axon_stop_nrt_profile.argtypes = [ctypes.c_char_p]
        lib.axon_stop_nrt_profile.restype = ctypes.c_int64
    except (OSError, AttributeError):
        return

    @contextlib.contextmanager
    def _hook(output_dir, device_ids):
        import jax

        jax.devices()
        if device_ids:
            ids = (ctypes.c_int64 * len(device_ids))(*device_ids)
            rc = lib.axon_start_nrt_profile(ids, len(device_ids))
        else:
            rc = lib.axon_start_nrt_profile(None, 0)
        if rc != 0:
            raise RuntimeError(f"axon_start_nrt_profile rc={rc}")
        try:
            yield
        finally:
            n = lib.axon_stop_nrt_profile(str(output_dir).encode())
            if n < 0:
                raise RuntimeError(f"axon_stop_nrt_profile rc={n}")

    mod = types.ModuleType("antenv.axon_hooks")
    mod.get_axon_ntff_profile_hook = lambda: _hook
    mod.set_axon_ntff_profile_hook = lambda h: None
    sys.modules["antenv.axon_hooks"] = mod


def run(inputs, trace=False):
    """Run on the 8 NeuronCores. Returns (output, BassKernelResults)."""
    from concourse.bass_utils import run_bass_kernel_spmd

    if trace:
        _install_ntff_hook()
    nc = _get_program()
    in_maps = make_in_maps(**inputs)
    res = run_bass_kernel_spmd(
        nc, in_maps, core_ids=list(range(N_CORES)), trace=trace
    )
    return assemble(res.results), res


def kernel(x, Wqkv, bqkv, Wout, bout):
    out, _ = run(
        {"x": x, "Wqkv": Wqkv, "bqkv": bqkv, "Wout": Wout, "bout": bout},
        trace=False,
    )
    return out



# revision 6
# speedup vs baseline: 1.3360x; 1.3360x over previous
"""Trainium2 Bass kernel for naive causal MHA (dense transformer block).

Problem: x[2, 2048, 1024], per-head QKV (16 heads, head_dim 64), causal
softmax attention, concat heads, output projection.

Sharding (8 NeuronCores, tensor-parallel over heads):
  - core c computes QKV + attention for heads {2c, 2c+1} over both batches
    in a transposed layout: scores are built as [keys, queries] so the
    softmax denominator comes from an extra ones-column in V and the
    attention output lands directly in the [head_dim, seq] layout the
    output projection needs as its stationary operand.
  - one 8-way AllToAll PER BATCH reshards y from head-split to row-split
    (the batch-0 collective and output projection overlap batch-1 compute),
  - each core computes a disjoint 256-row slice of y @ Wout + bout per batch.

Perf notes vs the f32r baseline (450 us):
  - all matmuls in bf16 (f32r moving operands stream at half rate),
  - exp over [128, 4*512] groups spanning 4 PSUM banks (2 t-blocks x 2
    heads) to amortize the ~352-cycle ACTIVATE instruction overhead,
  - softmax normalization via reciprocal_approx_fast + a PE ones-broadcast
    matmul instead of a DVE iterative reciprocal + DRAM round-trip,
  - QKV(b1) / out-proj(b0) matmuls are interleaved into the ACT-bound
    attention instruction stream to fill PE bubbles.
"""

import contextlib
import ctypes
import sys
import types

import numpy as np

import concourse.bacc as bacc
import concourse.mybir as mybir
import concourse.tile as tile
from concourse.bass import ds

N_CORES = 8
B = 2
S = 2048
D = 1024
HD = 64
N_HEADS = 16

F32 = mybir.dt.float32
DTB = mybir.dt.bfloat16
NP_BF16 = mybir.dt.np(mybir.dt.bfloat16)

SC = 512          # seq chunk (moving-operand width)
N_SC = S // SC    # 4
N_DC = D // 128   # 8 contraction chunks
N_SB = S // 128   # 16 seq 128-blocks
CQ = S // N_CORES // B  # 128... no: per-batch a2a slot width = 2048/8 = 256
CQ = S // N_CORES       # 256 q per a2a slot


def _f32r(ap):
    return ap.bitcast(mybir.dt.float32r)


def _mask_np():
    """mask[X][t, slot, q] for the two diagonal exp groups of a q-chunk.
    Group X covers t-blocks (2X, 2X+1) relative to the diagonal; slots are
    (tb0,h0),(tb0,h1),(tb1,h0),(tb1,h1). m_j: q-subblocks < j are zero,
    == j upper-triangular (keep t <= q), > j ones."""
    tri = np.triu(np.ones((128, 128), dtype=np.float32))
    m = np.zeros((2, 128, 4, SC), dtype=np.float32)
    for X in range(2):
        for sl in range(4):
            j = 2 * X + sl // 2
            m[X, :, sl, j * 128 : (j + 1) * 128] = tri
            m[X, :, sl, (j + 1) * 128 :] = 1.0
    return m.astype(NP_BF16)


def _build_program():
    nc = bacc.Bacc(
        "TRN2", target_bir_lowering=False, debug=False, num_devices=N_CORES
    )

    xt_d = nc.dram_tensor("xt", [B, D, S], DTB, kind="ExternalInput").ap()
    wq_d = nc.dram_tensor("wq", [D, 128], DTB, kind="ExternalInput").ap()
    wk_d = nc.dram_tensor("wk", [D, 128], DTB, kind="ExternalInput").ap()
    wv_d = nc.dram_tensor("wv", [D, 128], DTB, kind="ExternalInput").ap()
    bq_d = nc.dram_tensor("bq", [128, 1], F32, kind="ExternalInput").ap()
    bk_d = nc.dram_tensor("bk", [128, 1], F32, kind="ExternalInput").ap()
    bv_d = nc.dram_tensor("bv", [1, 128], F32, kind="ExternalInput").ap()
    wout_d = nc.dram_tensor("wout", [D, D], DTB, kind="ExternalInput").ap()
    bout_d = nc.dram_tensor("bout", [1, D], F32, kind="ExternalInput").ap()
    out_d = nc.dram_tensor("out", [B, 2 * 128, D], F32, kind="ExternalOutput").ap()

    y_part = [nc.dram_tensor(f"y{b}p", [N_CORES, 128, CQ], DTB) for b in range(B)]
    y_all = [nc.dram_tensor(f"y{b}a", [N_CORES, 128, CQ], DTB) for b in range(B)]

    mask_d = nc.inline_tensor(_mask_np(), name="maskAB")
    ones64_d = nc.inline_tensor(np.ones((1, 64), dtype=NP_BF16), name="ones64")

    with tile.TileContext(nc) as tc, contextlib.ExitStack() as ctx:
        const = ctx.enter_context(tc.tile_pool(name="const", bufs=1))
        xt_pool = ctx.enter_context(tc.tile_pool(name="xt", bufs=1))
        qk_pool = ctx.enter_context(tc.tile_pool(name="qk", bufs=1))
        v_pool = ctx.enter_context(tc.tile_pool(name="vp", bufs=1))
        exp_pool = ctx.enter_context(tc.tile_pool(name="expp", bufs=2))
        r_pool = ctx.enter_context(tc.tile_pool(name="rp", bufs=2))
        z_pool = ctx.enter_context(tc.tile_pool(name="zp", bufs=2))
        yts_pool = ctx.enter_context(tc.tile_pool(name="yts", bufs=3))
        yg_pool = ctx.enter_context(tc.tile_pool(name="yg", bufs=1))
        outs_pool = ctx.enter_context(tc.tile_pool(name="outs", bufs=2))
        psum = ctx.enter_context(tc.tile_pool(name="psum", bufs=1, space="PSUM"))

        # ---- constants into SBUF ----
        wq_sb = const.tile([128, N_DC, 128], DTB)
        nc.sync.dma_start(out=wq_sb, in_=wq_d.rearrange("(c p) e -> p c e", p=128))
        wk_sb = const.tile([128, N_DC, 128], DTB)
        nc.sync.dma_start(out=wk_sb, in_=wk_d.rearrange("(c p) e -> p c e", p=128))
        wv_sb = const.tile([128, N_DC, 128], DTB)
        nc.sync.dma_start(out=wv_sb, in_=wv_d.rearrange("(c p) e -> p c e", p=128))
        wout_sb = const.tile([128, N_DC, D], DTB)
        nc.sync.dma_start(out=wout_sb, in_=wout_d.rearrange("(c p) e -> p c e", p=128))
        bq_sb = const.tile([128, 1], F32)
        nc.sync.dma_start(out=bq_sb, in_=bq_d)
        bk_sb = const.tile([128, 1], F32)
        nc.sync.dma_start(out=bk_sb, in_=bk_d)
        bv_bc = const.tile([128, 4, 128], F32)
        for j in range(4):
            nc.sync.dma_start(out=bv_bc[:, j, :], in_=bv_d.to_broadcast([128, 128]))
        bout_bc = const.tile([128, D], F32)
        nc.sync.dma_start(out=bout_bc, in_=bout_d.to_broadcast([128, D]))
        maskA_sb = const.tile([128, 4, SC], DTB)
        nc.sync.dma_start(out=maskA_sb, in_=mask_d.ap()[0])
        maskB_sb = const.tile([128, 4, SC], DTB)
        nc.sync.dma_start(out=maskB_sb, in_=mask_d.ap()[1])
        ones64_sb = const.tile([1, 64], DTB)
        nc.sync.dma_start(out=ones64_sb, in_=ones64_d.ap())

        # ---- per-batch persistent SBUF ----
        xt_sb = []
        qT, kT, qT1, kT1, v_sb = [], [], [], [], []
        for b in range(B):
            xt_sb.append(xt_pool.tile([128, N_DC, S], DTB, tag=f"xt{b}", name=f"xt{b}"))
            qT.append(qk_pool.tile([128, S], DTB, tag=f"qT{b}", name=f"qT{b}"))
            kT.append(qk_pool.tile([128, S], DTB, tag=f"kT{b}", name=f"kT{b}"))
            qT1.append(qk_pool.tile([64, S], DTB, tag=f"qT1{b}", name=f"qT1{b}"))
            kT1.append(qk_pool.tile([64, S], DTB, tag=f"kT1{b}", name=f"kT1{b}"))
            v_sb.append(v_pool.tile([128, N_SB, 2, 65], DTB, tag=f"v{b}", name=f"v{b}"))

        def load_xt(b):
            for dc in range(N_DC):
                nc.sync.dma_start(
                    out=xt_sb[b][:, dc, :], in_=xt_d[b, ds(dc * 128, 128), :]
                )

        def v_ones(b):
            nc.vector.memset(
                v_sb[b].rearrange("p a h e -> p (a h) e")[:, :, 64:65], 1.0
            )

        def qkv_q(b, sc):
            psq = psum.tile([128, SC], F32, tag="misc", bufs=2)
            for dc in range(N_DC):
                nc.tensor.matmul(
                    psq, wq_sb[:, dc, :], xt_sb[b][:, dc, ds(sc * SC, SC)],
                    start=(dc == 0), stop=(dc == N_DC - 1),
                )
            nc.vector.tensor_scalar_add(
                out=qT[b][:, ds(sc * SC, SC)], in0=psq, scalar1=bq_sb
            )
            nc.sync.dma_start(
                out=qT1[b][:, ds(sc * SC, SC)], in_=qT[b][64:128, ds(sc * SC, SC)]
            )

        def qkv_k(b, sc):
            psk = psum.tile([128, SC], F32, tag="misc", bufs=2)
            for dc in range(N_DC):
                nc.tensor.matmul(
                    psk, wk_sb[:, dc, :], xt_sb[b][:, dc, ds(sc * SC, SC)],
                    start=(dc == 0), stop=(dc == N_DC - 1),
                )
            nc.vector.tensor_scalar_add(
                out=kT[b][:, ds(sc * SC, SC)], in0=psk, scalar1=bk_sb
            )
            nc.sync.dma_start(
                out=kT1[b][:, ds(sc * SC, SC)], in_=kT[b][64:128, ds(sc * SC, SC)]
            )

        def qkv_v(b, sc):
            psv = psum.tile([128, SC], F32, tag="misc", bufs=2)
            psv4 = psv.rearrange("p (j e) -> p j e", j=4)
            for j4 in range(4):
                for dc in range(N_DC):
                    nc.tensor.matmul(
                        psv4[:, j4, :],
                        xt_sb[b][:, dc, ds(sc * SC + j4 * 128, 128)],
                        wv_sb[:, dc, :],
                        start=(dc == 0), stop=(dc == N_DC - 1),
                    )
            pjhe = psv.rearrange("p (j h e) -> p j h e", j=4, h=2)
            bjhe = bv_bc.rearrange("p j (h e) -> p j h e", h=2)
            for h in range(2):
                nc.vector.tensor_add(
                    out=v_sb[b][:, ds(4 * sc, 4), h, 0:64],
                    in0=pjhe[:, :, h, :],
                    in1=bjhe[:, :, h, :],
                )

        # slot order within an exp group: (tb0,h0),(tb0,h1),(tb1,h0),(tb1,h1)
        SLOTS = [(0, 0), (0, 1), (1, 0), (1, 1)]

        def attn_qc(b, qc, fillers):
            ngrp = 2 * qc + 2
            psy = [
                psum.tile([65, SC], F32, tag="psy", bufs=2, name=f"psy{b}_{qc}_{_}") for _ in range(2)
            ]
            for g in range(ngrp):
                psc = psum.tile([128, 4, SC], F32, tag="sc4", bufs=1)
                ex = exp_pool.tile([128, 4, SC], DTB)
                for s, (dt, h) in enumerate(SLOTS):
                    tb = 2 * g + dt
                    qTh, kTh = (qT[b], kT[b]) if h == 0 else (qT1[b], kT1[b])
                    nc.tensor.matmul(
                        psc[:, s, :],
                        kTh[0:64, ds(tb * 128, 128)],
                        qTh[0:64, ds(qc * SC, SC)],
                        start=True, stop=True,
                    )
                nc.scalar.activation(
                    out=ex, in_=psc,
                    func=mybir.ActivationFunctionType.Exp,
                    scale=0.125,
                )
                if g == ngrp - 2:
                    nc.vector.tensor_mul(out=ex, in0=ex, in1=maskA_sb)
                elif g == ngrp - 1:
                    nc.vector.tensor_mul(out=ex, in0=ex, in1=maskB_sb)
                for s, (dt, h) in enumerate(SLOTS):
                    tb = 2 * g + dt
                    nc.tensor.matmul(
                        psy[h], v_sb[b][:, tb, h, :], ex[:, s, :],
                        start=(g == 0 and dt == 0),
                        stop=(g == ngrp - 1 and dt == 1),
                    )
                if fillers:
                    fillers.pop(0)()
            # normalize + emit y_part slots
            for h in range(2):
                zrow = r_pool.tile([1, SC], DTB)
                nc.vector.tensor_copy(out=zrow, in_=psy[h][64:65, :])
                zb = psum.tile([128, SC], F32, tag="misc", bufs=2)
                nc.tensor.matmul(
                    zb[0:64, :], ones64_sb, zrow, start=True, stop=True
                )
                rbc = z_pool.tile([64, SC], F32)
                nc.vector.reciprocal_approx_fast(out=rbc, in_=zb[0:64, :])
                yts = yts_pool.tile([64, SC], DTB)
                nc.vector.tensor_mul(out=yts, in0=psy[h][0:64, :], in1=rbc)
                nc.sync.dma_start(
                    out=y_part[b].ap()[ds(2 * qc, 2), ds(64 * h, 64), :].transpose(
                        [1, 0, 2]
                    ),
                    in_=yts.rearrange("p (c q) -> p c q", c=2),
                )

        ygs = [yg_pool.tile([128, N_CORES, CQ], DTB, tag=f"yg{b}", name=f"yg{b}") for b in range(B)]

        def load_ygs(b):
            nc.sync.dma_start(
                out=ygs[b], in_=y_all[b].ap().transpose([1, 0, 2])
            )

        def outproj_piece(b, qb, ch):
            pso = psum.tile([128, SC], F32, tag="misc", bufs=2)
            for ec in range(N_CORES):
                nc.tensor.matmul(
                    pso,
                    ygs[b][:, ec, ds(qb * 128, 128)],
                    wout_sb[:, ec, ds(ch * SC, SC)],
                    start=(ec == 0), stop=(ec == N_CORES - 1),
                )
            ot = outs_pool.tile([128, SC], F32)
            nc.vector.tensor_add(out=ot, in0=pso, in1=bout_bc[:, ds(ch * SC, SC)])
            nc.sync.dma_start(
                out=out_d[b, ds(qb * 128, 128), ds(ch * SC, SC)], in_=ot
            )

        def a2a(b):
            nc.gpsimd.collective_compute(
                "AllToAll",
                mybir.AluOpType.bypass,
                replica_groups=[list(range(N_CORES))],
                ins=[y_part[b].ap()],
                outs=[y_all[b].ap()],
            )

        # ================= emission =================
        load_xt(0)
        v_ones(0)
        v_ones(1)
        for sc in range(N_SC):
            qkv_q(0, sc)
            qkv_k(0, sc)
            qkv_v(0, sc)
        load_xt(1)

        # attention(b0), with QKV(b1) matmuls as PE-bubble fillers
        fillers = []
        for sc in range(N_SC):
            fillers += [
                lambda sc=sc: qkv_q(1, sc),
                lambda sc=sc: qkv_k(1, sc),
                lambda sc=sc: qkv_v(1, sc),
            ]
        for qc in range(N_SC):
            attn_qc(0, qc, fillers)
        for f in fillers:
            f()

        a2a(0)

        # attention(b1), with out-proj(b0) as fillers once a2a(0) has landed
        fillers2 = []
        for qc in range(N_SC):
            if qc == 2:
                fillers2 = [lambda: load_ygs(0)] + [
                    lambda qb=qb, ch=ch: outproj_piece(0, qb, ch)
                    for qb in range(2)
                    for ch in range(2)
                ]
            attn_qc(1, qc, fillers2)
        for f in fillers2:
            f()

        a2a(1)
        load_ygs(1)
        for qb in range(2):
            for ch in range(2):
                outproj_piece(1, qb, ch)

    nc.compile()
    return nc


_NC_CACHE = None


def _get_program():
    global _NC_CACHE
    if _NC_CACHE is None:
        _NC_CACHE = _build_program()
    return _NC_CACHE


def make_in_maps(x, Wqkv, bqkv, Wout, bout):
    x = np.asarray(x, dtype=np.float32)
    Wqkv = np.asarray(Wqkv, dtype=np.float32)
    bqkv = np.asarray(bqkv, dtype=np.float32)
    Wout = np.asarray(Wout, dtype=np.float32)
    bout = np.asarray(bout, dtype=np.float32)

    xt = np.ascontiguousarray(x.transpose(0, 2, 1)).astype(NP_BF16)  # [B, D, S]
    wout = np.ascontiguousarray(Wout).astype(NP_BF16)
    bout2 = np.ascontiguousarray(bout.reshape(1, D))

    in_maps = []
    for c in range(N_CORES):
        h0, h1 = 2 * c, 2 * c + 1
        wq = np.concatenate(
            [Wqkv[h0, :, 0:64], Wqkv[h1, :, 0:64]], axis=1
        ).astype(NP_BF16)
        wk = np.concatenate(
            [Wqkv[h0, :, 64:128], Wqkv[h1, :, 64:128]], axis=1
        ).astype(NP_BF16)
        wv = np.concatenate(
            [Wqkv[h0, :, 128:192], Wqkv[h1, :, 128:192]], axis=1
        ).astype(NP_BF16)
        bq = np.ascontiguousarray(
            np.concatenate([bqkv[h0, 0:64], bqkv[h1, 0:64]]).reshape(128, 1)
        )
        bk = np.ascontiguousarray(
            np.concatenate([bqkv[h0, 64:128], bqkv[h1, 64:128]]).reshape(128, 1)
        )
        bv = np.ascontiguousarray(
            np.concatenate([bqkv[h0, 128:192], bqkv[h1, 128:192]]).reshape(1, 128)
        )
        in_maps.append(
            {
                "xt": xt,
                "wq": np.ascontiguousarray(wq),
                "wk": np.ascontiguousarray(wk),
                "wv": np.ascontiguousarray(wv),
                "bq": bq,
                "bk": bk,
                "bv": bv,
                "wout": wout,
                "bout": bout2,
            }
        )
    return in_maps


def assemble(results):
    full = np.empty((B, S, D), dtype=np.float32)
    for c in range(N_CORES):
        full[:, 256 * c : 256 * (c + 1)] = results[c]["out"]
    return full


def _install_ntff_hook():
    """The agent image's antenv lacks axon_hooks; provide it so
    run_bass_kernel_spmd(trace=True) can NTFF-profile via libaxon."""
    if "antenv.axon_hooks" in sys.modules:
        return
    so_path = "/opt/axon/libaxon_pjrt.so"
    try:
        lib = ctypes.CDLL(so_path)
        lib.axon_start_nrt_profile.argtypes = [
            ctypes.POINTER(ctypes.c_int64),
            ctypes.c_size_t,
        ]
        lib.axon_start_nrt_profile.restype = ctypes.c_int64
        lib.axon_stop_nrt_profile.argtypes = [ctypes.c_char_p]
        lib.axon_stop_nrt_profile.restype = ctypes.c_int64
    except (OSError, AttributeError):
        return

    @contextlib.contextmanager
    def _hook(output_dir, device_ids):
        import jax

        jax.devices()
        if device_ids:
            ids = (ctypes.c_int64 * len(device_ids))(*device_ids)
            rc = lib.axon_start_nrt_profile(ids, len(device_ids))
        else:
            rc = lib.axon_start_nrt_profile(None, 0)
        if rc != 0:
            raise RuntimeError(f"axon_start_nrt_profile rc={rc}")
        try:
            yield
        finally:
            n = lib.axon_stop_nrt_profile(str(output_dir).encode())
            if n < 0:
                raise RuntimeError(f"axon_stop_nrt_profile rc={n}")

    mod = types.ModuleType("antenv.axon_hooks")
    mod.get_axon_ntff_profile_hook = lambda: _hook
    mod.set_axon_ntff_profile_hook = lambda h: None
    sys.modules["antenv.axon_hooks"] = mod


def run(inputs, trace=False):
    """Run on the 8 NeuronCores. Returns (output, BassKernelResults)."""
    from concourse.bass_utils import run_bass_kernel_spmd

    if trace:
        _install_ntff_hook()
    nc = _get_program()
    in_maps = make_in_maps(**inputs)
    res = run_bass_kernel_spmd(
        nc, in_maps, core_ids=list(range(N_CORES)), trace=trace
    )
    return assemble(res.results), res


def kernel(x, Wqkv, bqkv, Wout, bout):
    out, _ = run(
        {"x": x, "Wqkv": Wqkv, "bqkv": bqkv, "Wout": Wout, "bout": bout},
        trace=False,
    )
    return out


# revision 9
# speedup vs baseline: 1.4639x; 1.0957x over previous
"""Trainium2 Bass kernel for naive causal MHA (dense transformer block).

Problem: x[2, 2048, 1024], per-head QKV (16 heads, head_dim 64), causal
softmax attention, concat heads, output projection.

Sharding (8 NeuronCores, tensor-parallel over heads):
  - core c computes QKV + attention for heads {2c, 2c+1} over both batches
    in a transposed layout: scores are built as [keys, queries] so the
    softmax denominator comes from an extra ones-column in V and the
    attention output lands directly in the [head_dim, seq] layout the
    output projection needs as its stationary operand.
  - one 8-way AllToAll PER BATCH reshards y from head-split to row-split
    (the batch-0 collective and output projection overlap batch-1 compute),
  - each core computes a disjoint 256-row slice of y @ Wout + bout per batch.

Perf notes vs the f32r baseline (450 us):
  - all matmuls in bf16 (f32r moving operands stream at half rate),
  - exp over [128, 4*512] groups spanning 4 PSUM banks (2 t-blocks x 2
    heads) to amortize the ~352-cycle ACTIVATE instruction overhead,
  - softmax normalization via reciprocal_approx_fast + a PE ones-broadcast
    matmul instead of a DVE iterative reciprocal + DRAM round-trip,
  - QKV(b1) / out-proj(b0) matmuls are interleaved into the ACT-bound
    attention instruction stream to fill PE bubbles.
"""

import contextlib
import ctypes
import sys
import types

import numpy as np

import concourse.bacc as bacc
import concourse.mybir as mybir
import concourse.tile as tile
from concourse.bass import ds

N_CORES = 8
B = 2
S = 2048
D = 1024
HD = 64
N_HEADS = 16

F32 = mybir.dt.float32
DTB = mybir.dt.bfloat16
NP_BF16 = mybir.dt.np(mybir.dt.bfloat16)

SC = 512          # seq chunk (moving-operand width)
N_SC = S // SC    # 4
N_DC = D // 128   # 8 contraction chunks
N_SB = S // 128   # 16 seq 128-blocks
CQ = S // N_CORES // B  # 128... no: per-batch a2a slot width = 2048/8 = 256
CQ = S // N_CORES       # 256 q per a2a slot


def _f32r(ap):
    return ap.bitcast(mybir.dt.float32r)


def _mask_np():
    """Upper-triangular keep-mask (t <= q) for the diagonal 128x128 score
    block, duplicated for both heads: [t, h, q]."""
    tri = np.triu(np.ones((128, 128), dtype=np.float32))
    return np.stack([tri, tri], axis=1).astype(NP_BF16)


def _build_program():
    nc = bacc.Bacc(
        "TRN2", target_bir_lowering=False, debug=False, num_devices=N_CORES
    )

    xt_d = nc.dram_tensor("xt", [B, D, S], DTB, kind="ExternalInput").ap()
    wq_d = nc.dram_tensor("wq", [D, 128], DTB, kind="ExternalInput").ap()
    wk_d = nc.dram_tensor("wk", [D, 128], DTB, kind="ExternalInput").ap()
    wv_d = nc.dram_tensor("wv", [D, 128], DTB, kind="ExternalInput").ap()
    bq_d = nc.dram_tensor("bq", [128, 1], F32, kind="ExternalInput").ap()
    bk_d = nc.dram_tensor("bk", [128, 1], F32, kind="ExternalInput").ap()
    bv_d = nc.dram_tensor("bv", [1, 128], F32, kind="ExternalInput").ap()
    wout_d = nc.dram_tensor("wout", [D, D], DTB, kind="ExternalInput").ap()
    bout_d = nc.dram_tensor("bout", [1, D], F32, kind="ExternalInput").ap()
    out_d = nc.dram_tensor("out", [B, 2 * 128, D], F32, kind="ExternalOutput").ap()

    y_part = [nc.dram_tensor(f"y{b}p", [N_CORES, 128, CQ], DTB) for b in range(B)]
    y_all = [nc.dram_tensor(f"y{b}a", [N_CORES, 128, CQ], DTB) for b in range(B)]

    mask_d = nc.inline_tensor(_mask_np(), name="tri")
    ones64_d = nc.inline_tensor(np.ones((1, 64), dtype=NP_BF16), name="ones64")

    with tile.TileContext(nc) as tc, contextlib.ExitStack() as ctx:
        const = ctx.enter_context(tc.tile_pool(name="const", bufs=1))
        xt_pool = ctx.enter_context(tc.tile_pool(name="xt", bufs=1))
        qk_pool = ctx.enter_context(tc.tile_pool(name="qk", bufs=1))
        v_pool = ctx.enter_context(tc.tile_pool(name="vp", bufs=1))
        exp_pool = ctx.enter_context(tc.tile_pool(name="expp", bufs=2))
        r_pool = ctx.enter_context(tc.tile_pool(name="rp", bufs=2))
        z_pool = ctx.enter_context(tc.tile_pool(name="zp", bufs=2))
        yts_pool = ctx.enter_context(tc.tile_pool(name="yts", bufs=3))
        yg_pool = ctx.enter_context(tc.tile_pool(name="yg", bufs=1))
        outs_pool = ctx.enter_context(tc.tile_pool(name="outs", bufs=2))
        psum = ctx.enter_context(tc.tile_pool(name="psum", bufs=1, space="PSUM"))

        # ---- constants into SBUF ----
        wq_sb = const.tile([128, N_DC, 128], DTB)
        nc.sync.dma_start(out=wq_sb, in_=wq_d.rearrange("(c p) e -> p c e", p=128))
        wk_sb = const.tile([128, N_DC, 128], DTB)
        nc.sync.dma_start(out=wk_sb, in_=wk_d.rearrange("(c p) e -> p c e", p=128))
        wv_sb = const.tile([128, N_DC, 128], DTB)
        nc.sync.dma_start(out=wv_sb, in_=wv_d.rearrange("(c p) e -> p c e", p=128))
        wout_sb = const.tile([128, N_DC, D], DTB)
        bq_sb = const.tile([128, 1], F32)
        nc.sync.dma_start(out=bq_sb, in_=bq_d)
        bk_sb = const.tile([128, 1], F32)
        nc.sync.dma_start(out=bk_sb, in_=bk_d)
        bv_bc = const.tile([128, 4, 128], F32)
        for j in range(4):
            nc.sync.dma_start(out=bv_bc[:, j, :], in_=bv_d.to_broadcast([128, 128]))
        bout_bc = const.tile([128, D], F32)
        tri_sb = const.tile([128, 2, 128], DTB)
        nc.sync.dma_start(out=tri_sb, in_=mask_d.ap())
        ones64_sb = const.tile([1, 64], DTB)
        nc.sync.dma_start(out=ones64_sb, in_=ones64_d.ap())

        # ---- per-batch persistent SBUF ----
        xt_sb = []
        qT, kT, qT1, kT1, v_sb = [], [], [], [], []
        for b in range(B):
            xt_sb.append(xt_pool.tile([128, N_DC, S], DTB, tag=f"xt{b}", name=f"xt{b}"))
            qT.append(qk_pool.tile([128, S], DTB, tag=f"qT{b}", name=f"qT{b}"))
            kT.append(qk_pool.tile([128, S], DTB, tag=f"kT{b}", name=f"kT{b}"))
            qT1.append(qk_pool.tile([64, S], DTB, tag=f"qT1{b}", name=f"qT1{b}"))
            kT1.append(qk_pool.tile([64, S], DTB, tag=f"kT1{b}", name=f"kT1{b}"))
            v_sb.append(v_pool.tile([128, N_SB, 2, 65], DTB, tag=f"v{b}", name=f"v{b}"))

        def load_xt_sc(b, sc):
            for dc in range(N_DC):
                nc.sync.dma_start(
                    out=xt_sb[b][:, dc, ds(sc * SC, SC)],
                    in_=xt_d[b, ds(dc * 128, 128), ds(sc * SC, SC)],
                )

        def v_ones(b):
            nc.vector.memset(
                v_sb[b].rearrange("p a h e -> p (a h) e")[:, :, 64:65], 1.0
            )

        def qkv_q(b, sc):
            psq = psum.tile([128, SC], F32, tag="misc", bufs=2)
            for dc in range(N_DC):
                nc.tensor.matmul(
                    psq, wq_sb[:, dc, :], xt_sb[b][:, dc, ds(sc * SC, SC)],
                    start=(dc == 0), stop=(dc == N_DC - 1),
                )
            nc.vector.tensor_scalar_add(
                out=qT[b][:, ds(sc * SC, SC)], in0=psq, scalar1=bq_sb
            )
            nc.sync.dma_start(
                out=qT1[b][:, ds(sc * SC, SC)], in_=qT[b][64:128, ds(sc * SC, SC)]
            )

        def qkv_k(b, sc):
            psk = psum.tile([128, SC], F32, tag="misc", bufs=2)
            for dc in range(N_DC):
                nc.tensor.matmul(
                    psk, wk_sb[:, dc, :], xt_sb[b][:, dc, ds(sc * SC, SC)],
                    start=(dc == 0), stop=(dc == N_DC - 1),
                )
            nc.vector.tensor_scalar_add(
                out=kT[b][:, ds(sc * SC, SC)], in0=psk, scalar1=bk_sb
            )
            nc.sync.dma_start(
                out=kT1[b][:, ds(sc * SC, SC)], in_=kT[b][64:128, ds(sc * SC, SC)]
            )

        def qkv_v(b, sc):
            psv = psum.tile([128, SC], F32, tag="misc", bufs=2)
            psv4 = psv.rearrange("p (j e) -> p j e", j=4)
            for j4 in range(4):
                for dc in range(N_DC):
                    nc.tensor.matmul(
                        psv4[:, j4, :],
                        xt_sb[b][:, dc, ds(sc * SC + j4 * 128, 128)],
                        wv_sb[:, dc, :],
                        start=(dc == 0), stop=(dc == N_DC - 1),
                    )
            pjhe = psv.rearrange("p (j h e) -> p j h e", j=4, h=2)
            bjhe = bv_bc.rearrange("p j (h e) -> p j h e", h=2)
            for h in range(2):
                nc.vector.tensor_add(
                    out=v_sb[b][:, ds(4 * sc, 4), h, 0:64],
                    in0=pjhe[:, :, h, :],
                    in1=bjhe[:, :, h, :],
                )

        def attn_qc(b, qc, pop_filler):
            ntb = 4 * qc + 4
            psy = [
                psum.tile([65, SC], F32, tag="psy", bufs=2, name=f"psy{b}_{qc}_{_}")
                for _ in range(2)
            ]
            for tb in range(ntb):
                psc = psum.tile([128, 2, SC], F32, tag="sc2", bufs=2)
                ex = exp_pool.tile([128, 2, SC], DTB)
                for h in range(2):
                    qTh, kTh = (qT[b], kT[b]) if h == 0 else (qT1[b], kT1[b])
                    nc.tensor.matmul(
                        psc[:, h, :],
                        kTh[0:64, ds(tb * 128, 128)],
                        qTh[0:64, ds(qc * SC, SC)],
                        start=True, stop=True,
                    )
                pop_filler()
                nc.scalar.activation(
                    out=ex, in_=psc,
                    func=mybir.ActivationFunctionType.Exp,
                    scale=0.125,
                )
                j = tb - 4 * qc
                if j >= 0:
                    if j > 0:
                        nc.vector.memset(ex[:, :, 0 : j * 128], 0.0)
                    nc.vector.tensor_mul(
                        out=ex[:, :, ds(j * 128, 128)],
                        in0=ex[:, :, ds(j * 128, 128)],
                        in1=tri_sb,
                    )
                for h in range(2):
                    nc.tensor.matmul(
                        psy[h], v_sb[b][:, tb, h, :], ex[:, h, :],
                        start=(tb == 0), stop=(tb == ntb - 1),
                    )
            # normalize + emit y_part slots
            for h in range(2):
                zrow = r_pool.tile([1, SC], DTB)
                nc.vector.tensor_copy(out=zrow, in_=psy[h][64:65, :])
                zb = psum.tile([128, SC], F32, tag="misc", bufs=2)
                nc.tensor.matmul(
                    zb[0:64, :], ones64_sb, zrow, start=True, stop=True
                )
                rbc = z_pool.tile([64, SC], F32)
                nc.vector.reciprocal_approx_fast(out=rbc, in_=zb[0:64, :])
                yts = yts_pool.tile([64, SC], DTB)
                nc.vector.tensor_mul(out=yts, in0=psy[h][0:64, :], in1=rbc)
                nc.sync.dma_start(
                    out=y_part[b].ap()[ds(2 * qc, 2), ds(64 * h, 64), :].transpose(
                        [1, 0, 2]
                    ),
                    in_=yts.rearrange("p (c q) -> p c q", c=2),
                )

        ygs = [yg_pool.tile([128, N_CORES, CQ], DTB, tag=f"yg{b}", name=f"yg{b}") for b in range(B)]

        def load_ygs(b):
            nc.sync.dma_start(
                out=ygs[b], in_=y_all[b].ap().transpose([1, 0, 2])
            )

        def outproj_piece(b, qb, ch):
            pso = psum.tile([128, SC], F32, tag="misc", bufs=2)
            for ec in range(N_CORES):
                nc.tensor.matmul(
                    pso,
                    ygs[b][:, ec, ds(qb * 128, 128)],
                    wout_sb[:, ec, ds(ch * SC, SC)],
                    start=(ec == 0), stop=(ec == N_CORES - 1),
                )
            ot = outs_pool.tile([128, SC], F32)
            nc.vector.tensor_add(out=ot, in0=pso, in1=bout_bc[:, ds(ch * SC, SC)])
            nc.sync.dma_start(
                out=out_d[b, ds(qb * 128, 128), ds(ch * SC, SC)], in_=ot
            )

        def a2a(b):
            nc.gpsimd.collective_compute(
                "AllToAll",
                mybir.AluOpType.bypass,
                replica_groups=[list(range(N_CORES))],
                ins=[y_part[b].ap()],
                outs=[y_all[b].ap()],
            )

        # ================= emission =================
        # xt(b0) first (sc-major so QKV(b0,sc0) can start early)
        for sc in range(N_SC):
            load_xt_sc(0, sc)
        v_ones(0)
        v_ones(1)
        qkv_q(0, 0)
        qkv_k(0, 0)
        qkv_v(0, 0)
        for sc in range(N_SC):
            load_xt_sc(1, sc)
        nc.sync.dma_start(
            out=wout_sb, in_=wout_d.rearrange("(c p) e -> p c e", p=128)
        )
        nc.sync.dma_start(out=bout_bc, in_=bout_d.to_broadcast([128, D]))

        # remaining QKV work as an ordered unit queue; units with key
        # (b, sc) must be emitted before attn_qc(b, qc >= sc)
        units = []
        for key in [(0, 1), (0, 2), (0, 3), (1, 0), (1, 1), (1, 2), (1, 3)]:
            b, sc = key
            units.append((key, lambda b=b, sc=sc: qkv_q(b, sc)))
            units.append((key, lambda b=b, sc=sc: qkv_k(b, sc)))
            units.append((key, lambda b=b, sc=sc: qkv_v(b, sc)))

        def flush_to(key):
            while units and units[0][0] <= key:
                units.pop(0)[1]()

        def popper(limit_key):
            def pop():
                if units and units[0][0] <= limit_key:
                    units.pop(0)[1]()
            return pop

        for qc in range(N_SC):
            flush_to((0, qc))
            attn_qc(0, qc, popper((1, 0)))
        flush_to((1, 0))
        a2a(0)

        for qc in range(N_SC):
            flush_to((1, qc))
            attn_qc(1, qc, popper((1, 3)))
        a2a(1)

        load_ygs(0)
        for qb in range(2):
            for ch in range(2):
                outproj_piece(0, qb, ch)
        load_ygs(1)
        for qb in range(2):
            for ch in range(2):
                outproj_piece(1, qb, ch)

    nc.compile()
    return nc


_NC_CACHE = None


def _get_program():
    global _NC_CACHE
    if _NC_CACHE is None:
        _NC_CACHE = _build_program()
    return _NC_CACHE


def make_in_maps(x, Wqkv, bqkv, Wout, bout):
    x = np.asarray(x, dtype=np.float32)
    Wqkv = np.asarray(Wqkv, dtype=np.float32)
    bqkv = np.asarray(bqkv, dtype=np.float32)
    Wout = np.asarray(Wout, dtype=np.float32)
    bout = np.asarray(bout, dtype=np.float32)

    xt = np.ascontiguousarray(x.transpose(0, 2, 1)).astype(NP_BF16)  # [B, D, S]
    wout = np.ascontiguousarray(Wout).astype(NP_BF16)
    bout2 = np.ascontiguousarray(bout.reshape(1, D))

    in_maps = []
    for c in range(N_CORES):
        h0, h1 = 2 * c, 2 * c + 1
        wq = np.concatenate(
            [Wqkv[h0, :, 0:64], Wqkv[h1, :, 0:64]], axis=1
        ).astype(NP_BF16)
        wk = np.concatenate(
            [Wqkv[h0, :, 64:128], Wqkv[h1, :, 64:128]], axis=1
        ).astype(NP_BF16)
        wv = np.concatenate(
            [Wqkv[h0, :, 128:192], Wqkv[h1, :, 128:192]], axis=1
        ).astype(NP_BF16)
        bq = np.ascontiguousarray(
            np.concatenate([bqkv[h0, 0:64], bqkv[h1, 0:64]]).reshape(128, 1)
        )
        bk = np.ascontiguousarray(
            np.concatenate([bqkv[h0, 64:128], bqkv[h1, 64:128]]).reshape(128, 1)
        )
        bv = np.ascontiguousarray(
            np.concatenate([bqkv[h0, 128:192], bqkv[h1, 128:192]]).reshape(1, 128)
        )
        in_maps.append(
            {
                "xt": xt,
                "wq": np.ascontiguousarray(wq),
                "wk": np.ascontiguousarray(wk),
                "wv": np.ascontiguousarray(wv),
                "bq": bq,
                "bk": bk,
                "bv": bv,
                "wout": wout,
                "bout": bout2,
            }
        )
    return in_maps


def assemble(results):
    full = np.empty((B, S, D), dtype=np.float32)
    for c in range(N_CORES):
        full[:, 256 * c : 256 * (c + 1)] = results[c]["out"]
    return full


def _install_ntff_hook():
    """The agent image's antenv lacks axon_hooks; provide it so
    run_bass_kernel_spmd(trace=True) can NTFF-profile via libaxon."""
    if "antenv.axon_hooks" in sys.modules:
        return
    so_path = "/opt/axon/libaxon_pjrt.so"
    try:
        lib = ctypes.CDLL(so_path)
        lib.axon_start_nrt_profile.argtypes = [
            ctypes.POINTER(ctypes.c_int64),
            ctypes.c_size_t,
        ]
        lib.axon_start_nrt_profile.restype = ctypes.c_int64
        lib.axon_stop_nrt_profile.argtypes = [ctypes.c_char_p]
        lib.axon_stop_nrt_profile.restype = ctypes.c_int64
    except (OSError, AttributeError):
        return

    @contextlib.contextmanager
    def _hook(output_dir, device_ids):
        import jax

        jax.devices()
        if device_ids:
            ids = (ctypes.c_int64 * len(device_ids))(*device_ids)
            rc = lib.axon_start_nrt_profile(ids, len(device_ids))
        else:
            rc = lib.axon_start_nrt_profile(None, 0)
        if rc != 0:
            raise RuntimeError(f"axon_start_nrt_profile rc={rc}")
        try:
            yield
        finally:
            n = lib.axon_stop_nrt_profile(str(output_dir).encode())
            if n < 0:
                raise RuntimeError(f"axon_stop_nrt_profile rc={n}")

    mod = types.ModuleType("antenv.axon_hooks")
    mod.get_axon_ntff_profile_hook = lambda: _hook
    mod.set_axon_ntff_profile_hook = lambda h: None
    sys.modules["antenv.axon_hooks"] = mod


def run(inputs, trace=False):
    """Run on the 8 NeuronCores. Returns (output, BassKernelResults)."""
    from concourse.bass_utils import run_bass_kernel_spmd

    if trace:
        _install_ntff_hook()
    nc = _get_program()
    in_maps = make_in_maps(**inputs)
    res = run_bass_kernel_spmd(
        nc, in_maps, core_ids=list(range(N_CORES)), trace=trace
    )
    return assemble(res.results), res


def kernel(x, Wqkv, bqkv, Wout, bout):
    out, _ = run(
        {"x": x, "Wqkv": Wqkv, "bqkv": bqkv, "Wout": Wout, "bout": bout},
        trace=False,
    )
    return out


# revision 12
# speedup vs baseline: 1.6717x; 1.1419x over previous
"""Trainium2 Bass kernel for naive causal MHA (dense transformer block).

Problem: x[2, 2048, 1024], per-head QKV (16 heads, head_dim 64), causal
softmax attention, concat heads, output projection.

Sharding (8 NeuronCores, tensor-parallel over heads):
  - core c computes QKV + attention for heads {2c, 2c+1} over both batches
    in a transposed layout: scores are built as [keys, queries] so the
    softmax denominator comes from an extra ones-column in V and the
    attention output lands directly in the [head_dim, seq] layout the
    output projection needs as its stationary operand.
  - one 8-way AllToAll PER BATCH reshards y from head-split to row-split
    (the batch-0 collective and output projection overlap batch-1 compute),
  - each core computes a disjoint 256-row slice of y @ Wout + bout per batch.

Perf notes vs the f32r baseline (450 us):
  - all matmuls in bf16 (f32r moving operands stream at half rate),
  - exp over [128, 4*512] groups spanning 4 PSUM banks (2 t-blocks x 2
    heads) to amortize the ~352-cycle ACTIVATE instruction overhead,
  - softmax normalization via reciprocal_approx_fast + a PE ones-broadcast
    matmul instead of a DVE iterative reciprocal + DRAM round-trip,
  - QKV(b1) / out-proj(b0) matmuls are interleaved into the ACT-bound
    attention instruction stream to fill PE bubbles.
"""

import contextlib
import ctypes
import sys
import types

import numpy as np

import concourse.bacc as bacc
import concourse.mybir as mybir
import concourse.tile as tile
from concourse.bass import ds

N_CORES = 8
B = 2
S = 2048
D = 1024
HD = 64
N_HEADS = 16

F32 = mybir.dt.float32
DTB = mybir.dt.bfloat16
NP_BF16 = mybir.dt.np(mybir.dt.bfloat16)

SC = 512          # seq chunk (moving-operand width)
N_SC = S // SC    # 4
N_DC = D // 128   # 8 contraction chunks
N_SB = S // 128   # 16 seq 128-blocks
CQ = S // N_CORES // B  # 128... no: per-batch a2a slot width = 2048/8 = 256
CQ = S // N_CORES       # 256 q per a2a slot


def _f32r(ap):
    return ap.bitcast(mybir.dt.float32r)


def _mask_np():
    """Upper-triangular keep-mask (t <= q) for the diagonal 128x128 score
    block, duplicated for both heads: [t, h, q]."""
    tri = np.triu(np.ones((128, 128), dtype=np.float32))
    return np.stack([tri, tri], axis=1).astype(NP_BF16)


def _build_program():
    nc = bacc.Bacc(
        "TRN2", target_bir_lowering=False, debug=False, num_devices=N_CORES
    )

    xt_d = nc.dram_tensor("xt", [B, N_SC, N_DC, 128, SC], DTB, kind="ExternalInput").ap()
    wq_d = nc.dram_tensor("wq", [D, 128], DTB, kind="ExternalInput").ap()
    wk_d = nc.dram_tensor("wk", [D, 128], DTB, kind="ExternalInput").ap()
    wv_d = nc.dram_tensor("wv", [D, 128], DTB, kind="ExternalInput").ap()
    bq_d = nc.dram_tensor("bq", [128, 1], F32, kind="ExternalInput").ap()
    bk_d = nc.dram_tensor("bk", [128, 1], F32, kind="ExternalInput").ap()
    bv_d = nc.dram_tensor("bv", [1, 128], F32, kind="ExternalInput").ap()
    wout_d = nc.dram_tensor("wout", [D, D], DTB, kind="ExternalInput").ap()
    bout_d = nc.dram_tensor("bout", [1, D], F32, kind="ExternalInput").ap()
    out_d = nc.dram_tensor("out", [B, 2 * 128, D], F32, kind="ExternalOutput").ap()

    y_part = [nc.dram_tensor(f"y{b}p", [N_CORES, 128, CQ], DTB) for b in range(B)]
    y_all = [nc.dram_tensor(f"y{b}a", [N_CORES, 128, CQ], DTB) for b in range(B)]

    mask_d = nc.inline_tensor(_mask_np(), name="tri")
    ones64_d = nc.inline_tensor(np.ones((1, 64), dtype=NP_BF16), name="ones64")

    with tile.TileContext(nc) as tc, contextlib.ExitStack() as ctx:
        const = ctx.enter_context(tc.tile_pool(name="const", bufs=1))
        xt_pool = ctx.enter_context(tc.tile_pool(name="xt", bufs=1))
        qk_pool = ctx.enter_context(tc.tile_pool(name="qk", bufs=1))
        v_pool = ctx.enter_context(tc.tile_pool(name="vp", bufs=1))
        exp_pool = ctx.enter_context(tc.tile_pool(name="expp", bufs=2))
        r_pool = ctx.enter_context(tc.tile_pool(name="rp", bufs=2))
        z_pool = ctx.enter_context(tc.tile_pool(name="zp", bufs=2))
        yts_pool = ctx.enter_context(tc.tile_pool(name="yts", bufs=3))
        qs_pool = ctx.enter_context(tc.tile_pool(name="qs", bufs=2))
        yg_pool = ctx.enter_context(tc.tile_pool(name="yg", bufs=1))
        outs_pool = ctx.enter_context(tc.tile_pool(name="outs", bufs=2))
        psum = ctx.enter_context(tc.tile_pool(name="psum", bufs=1, space="PSUM"))

        # ---- constants into SBUF ----
        wq_sb = const.tile([128, N_DC, 128], DTB)
        nc.sync.dma_start(out=wq_sb, in_=wq_d.rearrange("(c p) e -> p c e", p=128))
        wk_sb = const.tile([128, N_DC, 128], DTB)
        nc.sync.dma_start(out=wk_sb, in_=wk_d.rearrange("(c p) e -> p c e", p=128))
        wv_sb = const.tile([128, N_DC, 128], DTB)
        nc.sync.dma_start(out=wv_sb, in_=wv_d.rearrange("(c p) e -> p c e", p=128))
        wout_sb = const.tile([128, N_DC, D], DTB)
        bq_sb = const.tile([128, 1], F32)
        nc.sync.dma_start(out=bq_sb, in_=bq_d)
        bk_sb = const.tile([128, 1], F32)
        nc.sync.dma_start(out=bk_sb, in_=bk_d)
        bv_bc = const.tile([128, 4, 128], F32)
        for j in range(4):
            nc.sync.dma_start(out=bv_bc[:, j, :], in_=bv_d.to_broadcast([128, 128]))
        bout_bc = const.tile([128, D], F32)
        tri_sb = const.tile([128, 2, 128], DTB)
        nc.sync.dma_start(out=tri_sb, in_=mask_d.ap())
        ones64_sb = const.tile([1, 64], DTB)
        nc.sync.dma_start(out=ones64_sb, in_=ones64_d.ap())

        # ---- per-batch persistent SBUF ----
        xt_sb = []
        qTz, kTz, v_sb = [], [], []
        for b in range(B):
            xt_sb.append(xt_pool.tile([128, N_DC, S], DTB, tag=f"xt{b}", name=f"xt{b}"))
            qTz.append([
                qk_pool.tile([128, S], DTB, tag=f"qTz{b}{h}", name=f"qTz{b}{h}")
                for h in range(2)
            ])
            kTz.append([
                qk_pool.tile([128, S], DTB, tag=f"kTz{b}{h}", name=f"kTz{b}{h}")
                for h in range(2)
            ])
            v_sb.append(v_pool.tile([128, N_SB, 2, 128], DTB, tag=f"v{b}", name=f"v{b}"))

        def zero_pads(b):
            # rows 64:128 of the per-head q/k tiles stay zero (full-width
            # contraction keeps the PE activity monitor at K=8/8); v cols
            # 64 = ones (softmax denominator), 65:128 = zero
            for h in range(2):
                nc.vector.memset(qTz[b][h][64:128, :], 0.0)
                nc.vector.memset(kTz[b][h][64:128, :], 0.0)
            nc.vector.memset(
                v_sb[b].rearrange("p a h e -> p (a h) e")[:, :, 64:65], 1.0
            )
            nc.vector.memset(
                v_sb[b].rearrange("p a h e -> p (a h) e")[:, :, 65:128], 0.0
            )

        def load_xt_sc(b, sc):
            for dc in range(N_DC):
                nc.sync.dma_start(
                    out=xt_sb[b][:, dc, ds(sc * SC, SC)],
                    in_=xt_d[b, sc, dc],
                )

        def _qk_proj(b, sc, w_sb, bias_sb, dstz, scratch_tag):
            ps = psum.tile([128, SC], F32, tag="misc", bufs=2, name=f"ps_{scratch_tag}")
            for dc in range(N_DC):
                nc.tensor.matmul(
                    ps, w_sb[:, dc, :], xt_sb[b][:, dc, ds(sc * SC, SC)],
                    start=(dc == 0), stop=(dc == N_DC - 1),
                )
            nc.vector.tensor_scalar_add(
                out=dstz[0][0:64, ds(sc * SC, SC)], in0=ps[0:64, :],
                scalar1=bias_sb[0:64],
            )
            qs = qs_pool.tile([128, SC], DTB, tag=scratch_tag, name=f"qs_{scratch_tag}")
            nc.vector.tensor_scalar_add(
                out=qs[64:128, :], in0=ps[64:128, :], scalar1=bias_sb[64:128]
            )
            nc.sync.dma_start(
                out=dstz[1][0:64, ds(sc * SC, SC)], in_=qs[64:128, :]
            )

        def qkv_q(b, sc):
            _qk_proj(b, sc, wq_sb, bq_sb, qTz[b], "qsq")

        def qkv_k(b, sc):
            _qk_proj(b, sc, wk_sb, bk_sb, kTz[b], "qsk")

        def qkv_v(b, sc):
            psv = psum.tile([128, SC], F32, tag="misc", bufs=2)
            psv4 = psv.rearrange("p (j e) -> p j e", j=4)
            for j4 in range(4):
                for dc in range(N_DC):
                    nc.tensor.matmul(
                        psv4[:, j4, :],
                        xt_sb[b][:, dc, ds(sc * SC + j4 * 128, 128)],
                        wv_sb[:, dc, :],
                        start=(dc == 0), stop=(dc == N_DC - 1),
                    )
            pjhe = psv.rearrange("p (j h e) -> p j h e", j=4, h=2)
            bjhe = bv_bc.rearrange("p j (h e) -> p j h e", h=2)
            for h in range(2):
                nc.vector.tensor_add(
                    out=v_sb[b][:, ds(4 * sc, 4), h, 0:64],
                    in0=pjhe[:, :, h, :],
                    in1=bjhe[:, :, h, :],
                )

        def attn_qc(b, qc, pop_filler):
            ntb = 4 * qc + 4
            psy = [
                psum.tile([128, SC], F32, tag="psy", bufs=2, name=f"psy{b}_{qc}_{_}")
                for _ in range(2)
            ]
            for tb in range(ntb):
                psc = psum.tile([128, 2, SC], F32, tag="sc2", bufs=2)
                ex = exp_pool.tile([128, 2, SC], DTB)
                for h in range(2):
                    nc.tensor.matmul(
                        psc[:, h, :],
                        kTz[b][h][:, ds(tb * 128, 128)],
                        qTz[b][h][:, ds(qc * SC, SC)],
                        start=True, stop=True,
                    )
                pop_filler()
                nc.scalar.activation(
                    out=ex, in_=psc,
                    func=mybir.ActivationFunctionType.Exp,
                    scale=0.125,
                )
                j = tb - 4 * qc
                if j >= 0:
                    if j > 0:
                        nc.vector.memset(ex[:, :, 0 : j * 128], 0.0)
                    nc.vector.tensor_mul(
                        out=ex[:, :, ds(j * 128, 128)],
                        in0=ex[:, :, ds(j * 128, 128)],
                        in1=tri_sb,
                    )
                for h in range(2):
                    nc.tensor.matmul(
                        psy[h], v_sb[b][:, tb, h, :], ex[:, h, :],
                        start=(tb == 0), stop=(tb == ntb - 1),
                    )
            # normalize + emit y_part slots
            for h in range(2):
                zrow = r_pool.tile([1, SC], DTB)
                nc.vector.tensor_copy(out=zrow, in_=psy[h][64:65, :])
                zb = psum.tile([128, SC], F32, tag="misc", bufs=2)
                nc.tensor.matmul(
                    zb[0:64, :], ones64_sb, zrow, start=True, stop=True
                )
                rbc = z_pool.tile([64, SC], F32)
                nc.vector.reciprocal_approx_fast(out=rbc, in_=zb[0:64, :])
                yts = yts_pool.tile([64, SC], DTB)
                nc.vector.tensor_mul(out=yts, in0=psy[h][0:64, :], in1=rbc)
                nc.sync.dma_start(
                    out=y_part[b].ap()[ds(2 * qc, 2), ds(64 * h, 64), :].transpose(
                        [1, 0, 2]
                    ),
                    in_=yts.rearrange("p (c q) -> p c q", c=2),
                )

        ygs = [yg_pool.tile([128, N_CORES, CQ], DTB, tag=f"yg{b}", name=f"yg{b}") for b in range(B)]

        def load_ygs(b):
            nc.sync.dma_start(
                out=ygs[b], in_=y_all[b].ap().transpose([1, 0, 2])
            )

        def outproj_piece(b, qb, ch):
            pso = psum.tile([128, SC], F32, tag="misc", bufs=2)
            for ec in range(N_CORES):
                nc.tensor.matmul(
                    pso,
                    ygs[b][:, ec, ds(qb * 128, 128)],
                    wout_sb[:, ec, ds(ch * SC, SC)],
                    start=(ec == 0), stop=(ec == N_CORES - 1),
                )
            ot = outs_pool.tile([128, SC], F32)
            nc.vector.tensor_add(out=ot, in0=pso, in1=bout_bc[:, ds(ch * SC, SC)])
            nc.sync.dma_start(
                out=out_d[b, ds(qb * 128, 128), ds(ch * SC, SC)], in_=ot
            )

        def a2a(b):
            nc.gpsimd.collective_compute(
                "AllToAll",
                mybir.AluOpType.bypass,
                replica_groups=[list(range(N_CORES))],
                ins=[y_part[b].ap()],
                outs=[y_all[b].ap()],
            )

        # ================= emission =================
        # xt(b0) first (sc-major so QKV(b0,sc0) can start early)
        for sc in range(N_SC):
            load_xt_sc(0, sc)
        zero_pads(0)
        zero_pads(1)
        qkv_q(0, 0)
        qkv_k(0, 0)
        qkv_v(0, 0)
        for sc in range(N_SC):
            load_xt_sc(1, sc)
        nc.sync.dma_start(
            out=wout_sb, in_=wout_d.rearrange("(c p) e -> p c e", p=128)
        )
        nc.sync.dma_start(out=bout_bc, in_=bout_d.to_broadcast([128, D]))

        # remaining QKV work as an ordered unit queue; units with key
        # (b, sc) must be emitted before attn_qc(b, qc >= sc)
        units = []
        for key in [(0, 1), (0, 2), (0, 3), (1, 0), (1, 1), (1, 2), (1, 3)]:
            b, sc = key
            units.append((key, lambda b=b, sc=sc: qkv_q(b, sc)))
            units.append((key, lambda b=b, sc=sc: qkv_k(b, sc)))
            units.append((key, lambda b=b, sc=sc: qkv_v(b, sc)))

        def flush_to(key):
            while units and units[0][0] <= key:
                units.pop(0)[1]()

        def popper(limit_key):
            def pop():
                if units and units[0][0] <= limit_key:
                    units.pop(0)[1]()
            return pop

        for qc in range(N_SC):
            flush_to((0, qc))
            attn_qc(0, qc, popper((1, 0)))
        flush_to((1, 0))
        a2a(0)

        for qc in range(N_SC):
            flush_to((1, qc))
            attn_qc(1, qc, popper((1, 3)))
        a2a(1)

        load_ygs(0)
        for qb in range(2):
            for ch in range(2):
                outproj_piece(0, qb, ch)
        load_ygs(1)
        for qb in range(2):
            for ch in range(2):
                outproj_piece(1, qb, ch)

    nc.compile()
    return nc


_NC_CACHE = None


def _get_program():
    global _NC_CACHE
    if _NC_CACHE is None:
        _NC_CACHE = _build_program()
    return _NC_CACHE


def make_in_maps(x, Wqkv, bqkv, Wout, bout):
    x = np.asarray(x, dtype=np.float32)
    Wqkv = np.asarray(Wqkv, dtype=np.float32)
    bqkv = np.asarray(bqkv, dtype=np.float32)
    Wout = np.asarray(Wout, dtype=np.float32)
    bout = np.asarray(bout, dtype=np.float32)

    # [B, D, S] -> chunk-major [B, N_SC, N_DC, 128, SC] for contiguous DMA
    xt = (
        x.transpose(0, 2, 1)
        .reshape(B, N_DC, 128, N_SC, SC)
        .transpose(0, 3, 1, 2, 4)
    )
    xt = np.ascontiguousarray(xt).astype(NP_BF16)
    wout = np.ascontiguousarray(Wout).astype(NP_BF16)
    bout2 = np.ascontiguousarray(bout.reshape(1, D))

    in_maps = []
    for c in range(N_CORES):
        h0, h1 = 2 * c, 2 * c + 1
        wq = np.concatenate(
            [Wqkv[h0, :, 0:64], Wqkv[h1, :, 0:64]], axis=1
        ).astype(NP_BF16)
        wk = np.concatenate(
            [Wqkv[h0, :, 64:128], Wqkv[h1, :, 64:128]], axis=1
        ).astype(NP_BF16)
        wv = np.concatenate(
            [Wqkv[h0, :, 128:192], Wqkv[h1, :, 128:192]], axis=1
        ).astype(NP_BF16)
        bq = np.ascontiguousarray(
            np.concatenate([bqkv[h0, 0:64], bqkv[h1, 0:64]]).reshape(128, 1)
        )
        bk = np.ascontiguousarray(
            np.concatenate([bqkv[h0, 64:128], bqkv[h1, 64:128]]).reshape(128, 1)
        )
        bv = np.ascontiguousarray(
            np.concatenate([bqkv[h0, 128:192], bqkv[h1, 128:192]]).reshape(1, 128)
        )
        in_maps.append(
            {
                "xt": xt,
                "wq": np.ascontiguousarray(wq),
                "wk": np.ascontiguousarray(wk),
                "wv": np.ascontiguousarray(wv),
                "bq": bq,
                "bk": bk,
                "bv": bv,
                "wout": wout,
                "bout": bout2,
            }
        )
    return in_maps


def assemble(results):
    full = np.empty((B, S, D), dtype=np.float32)
    for c in range(N_CORES):
        full[:, 256 * c : 256 * (c + 1)] = results[c]["out"]
    return full


def _install_ntff_hook():
    """The agent image's antenv lacks axon_hooks; provide it so
    run_bass_kernel_spmd(trace=True) can NTFF-profile via libaxon."""
    if "antenv.axon_hooks" in sys.modules:
        return
    so_path = "/opt/axon/libaxon_pjrt.so"
    try:
        lib = ctypes.CDLL(so_path)
        lib.axon_start_nrt_profile.argtypes = [
            ctypes.POINTER(ctypes.c_int64),
            ctypes.c_size_t,
        ]
        lib.axon_start_nrt_profile.restype = ctypes.c_int64
        lib.axon_stop_nrt_profile.argtypes = [ctypes.c_char_p]
        lib.axon_stop_nrt_profile.restype = ctypes.c_int64
    except (OSError, AttributeError):
        return

    @contextlib.contextmanager
    def _hook(output_dir, device_ids):
        import jax

        jax.devices()
        if device_ids:
            ids = (ctypes.c_int64 * len(device_ids))(*device_ids)
            rc = lib.axon_start_nrt_profile(ids, len(device_ids))
        else:
            rc = lib.axon_start_nrt_profile(None, 0)
        if rc != 0:
            raise RuntimeError(f"axon_start_nrt_profile rc={rc}")
        try:
            yield
        finally:
            n = lib.axon_stop_nrt_profile(str(output_dir).encode())
            if n < 0:
                raise RuntimeError(f"axon_stop_nrt_profile rc={n}")

    mod = types.ModuleType("antenv.axon_hooks")
    mod.get_axon_ntff_profile_hook = lambda: _hook
    mod.set_axon_ntff_profile_hook = lambda h: None
    sys.modules["antenv.axon_hooks"] = mod


def run(inputs, trace=False):
    """Run on the 8 NeuronCores. Returns (output, BassKernelResults)."""
    from concourse.bass_utils import run_bass_kernel_spmd

    if trace:
        _install_ntff_hook()
    nc = _get_program()
    in_maps = make_in_maps(**inputs)
    res = run_bass_kernel_spmd(
        nc, in_maps, core_ids=list(range(N_CORES)), trace=trace
    )
    return assemble(res.results), res


def kernel(x, Wqkv, bqkv, Wout, bout):
    out, _ = run(
        {"x": x, "Wqkv": Wqkv, "bqkv": bqkv, "Wout": Wout, "bout": bout},
        trace=False,
    )
    return out


# revision 14
# speedup vs baseline: 1.6813x; 1.0058x over previous
"""Trainium2 Bass kernel for naive causal MHA (dense transformer block).

Problem: x[2, 2048, 1024], per-head QKV (16 heads, head_dim 64), causal
softmax attention, concat heads, output projection.

Sharding (8 NeuronCores, tensor-parallel over heads):
  - core c computes QKV + attention for heads {2c, 2c+1} over both batches
    in a transposed layout: scores are built as [keys, queries] so the
    softmax denominator comes from an extra ones-column in V and the
    attention output lands directly in the [head_dim, seq] layout the
    output projection needs as its stationary operand.
  - one 8-way AllToAll PER BATCH reshards y from head-split to row-split
    (the batch-0 collective and output projection overlap batch-1 compute),
  - each core computes a disjoint 256-row slice of y @ Wout + bout per batch.

Perf notes vs the f32r baseline (450 us):
  - all matmuls in bf16 (f32r moving operands stream at half rate),
  - exp over [128, 4*512] groups spanning 4 PSUM banks (2 t-blocks x 2
    heads) to amortize the ~352-cycle ACTIVATE instruction overhead,
  - softmax normalization via reciprocal_approx_fast + a PE ones-broadcast
    matmul instead of a DVE iterative reciprocal + DRAM round-trip,
  - QKV(b1) / out-proj(b0) matmuls are interleaved into the ACT-bound
    attention instruction stream to fill PE bubbles.
"""

import contextlib
import ctypes
import sys
import types

import numpy as np

import concourse.bacc as bacc
import concourse.mybir as mybir
import concourse.tile as tile
from concourse.bass import ds

N_CORES = 8
B = 2
S = 2048
D = 1024
HD = 64
N_HEADS = 16

F32 = mybir.dt.float32
DTB = mybir.dt.bfloat16
NP_BF16 = mybir.dt.np(mybir.dt.bfloat16)

SC = 512          # seq chunk (moving-operand width)
N_SC = S // SC    # 4
N_DC = D // 128   # 8 contraction chunks
N_SB = S // 128   # 16 seq 128-blocks
CQ = S // N_CORES // B  # 128... no: per-batch a2a slot width = 2048/8 = 256
CQ = S // N_CORES       # 256 q per a2a slot


def _f32r(ap):
    return ap.bitcast(mybir.dt.float32r)


def _mask_np():
    """Upper-triangular keep-mask (t <= q) for the diagonal 128x128 score
    block, duplicated for both heads: [t, h, q]."""
    tri = np.triu(np.ones((128, 128), dtype=np.float32))
    return np.stack([tri, tri], axis=1).astype(NP_BF16)


def _build_program():
    nc = bacc.Bacc(
        "TRN2", target_bir_lowering=False, debug=False, num_devices=N_CORES
    )

    xt_d = nc.dram_tensor("xt", [B, N_SC, N_DC, 128, SC], DTB, kind="ExternalInput").ap()
    wq_d = nc.dram_tensor("wq", [D, 128], DTB, kind="ExternalInput").ap()
    wk_d = nc.dram_tensor("wk", [D, 128], DTB, kind="ExternalInput").ap()
    wv_d = nc.dram_tensor("wv", [D, 128], DTB, kind="ExternalInput").ap()
    bq_d = nc.dram_tensor("bq", [128, 1], F32, kind="ExternalInput").ap()
    bk_d = nc.dram_tensor("bk", [128, 1], F32, kind="ExternalInput").ap()
    bv_d = nc.dram_tensor("bv", [1, 128], F32, kind="ExternalInput").ap()
    wout_d = nc.dram_tensor("wout", [D, D], DTB, kind="ExternalInput").ap()
    bout_d = nc.dram_tensor("bout", [1, D], F32, kind="ExternalInput").ap()
    out_d = nc.dram_tensor("out", [B, 2 * 128, D], F32, kind="ExternalOutput").ap()

    y_part = [nc.dram_tensor(f"y{b}p", [N_CORES, 128, CQ], DTB) for b in range(B)]
    y_all = [nc.dram_tensor(f"y{b}a", [N_CORES, 128, CQ], DTB) for b in range(B)]

    mask_d = nc.inline_tensor(_mask_np(), name="tri")
    ones64_d = nc.inline_tensor(np.ones((1, 64), dtype=NP_BF16), name="ones64")

    with tile.TileContext(nc) as tc, contextlib.ExitStack() as ctx:
        const = ctx.enter_context(tc.tile_pool(name="const", bufs=1))
        xt_pool = ctx.enter_context(tc.tile_pool(name="xt", bufs=1))
        qk_pool = ctx.enter_context(tc.tile_pool(name="qk", bufs=1))
        v_pool = ctx.enter_context(tc.tile_pool(name="vp", bufs=1))
        exp_pool = ctx.enter_context(tc.tile_pool(name="expp", bufs=2))
        r_pool = ctx.enter_context(tc.tile_pool(name="rp", bufs=2))
        z_pool = ctx.enter_context(tc.tile_pool(name="zp", bufs=2))
        yts_pool = ctx.enter_context(tc.tile_pool(name="yts", bufs=3))
        qs_pool = ctx.enter_context(tc.tile_pool(name="qs", bufs=2))
        yg_pool = ctx.enter_context(tc.tile_pool(name="yg", bufs=1))
        outs_pool = ctx.enter_context(tc.tile_pool(name="outs", bufs=2))
        psum = ctx.enter_context(tc.tile_pool(name="psum", bufs=1, space="PSUM"))

        # ---- constants into SBUF ----
        wq_sb = const.tile([128, N_DC, 128], DTB)
        nc.sync.dma_start(out=wq_sb, in_=wq_d.rearrange("(c p) e -> p c e", p=128))
        wk_sb = const.tile([128, N_DC, 128], DTB)
        nc.sync.dma_start(out=wk_sb, in_=wk_d.rearrange("(c p) e -> p c e", p=128))
        wv_sb = const.tile([128, N_DC, 128], DTB)
        nc.sync.dma_start(out=wv_sb, in_=wv_d.rearrange("(c p) e -> p c e", p=128))
        wout_sb = const.tile([128, N_DC, D], DTB)
        bq_sb = const.tile([128, 1], F32)
        nc.sync.dma_start(out=bq_sb, in_=bq_d)
        bk_sb = const.tile([128, 1], F32)
        nc.sync.dma_start(out=bk_sb, in_=bk_d)
        bv_bc = const.tile([128, 4, 128], F32)
        for j in range(4):
            nc.sync.dma_start(out=bv_bc[:, j, :], in_=bv_d.to_broadcast([128, 128]))
        bout_bc = const.tile([128, D], F32)
        tri_sb = const.tile([128, 2, 128], DTB)
        nc.sync.dma_start(out=tri_sb, in_=mask_d.ap())
        ones64_sb = const.tile([1, 64], DTB)
        nc.sync.dma_start(out=ones64_sb, in_=ones64_d.ap())

        # ---- per-batch persistent SBUF ----
        xt_sb = []
        qTz, kTz, v_sb = [], [], []
        for b in range(B):
            xt_sb.append(xt_pool.tile([128, N_DC, S], DTB, tag=f"xt{b}", name=f"xt{b}"))
            qTz.append([
                qk_pool.tile([128, S], DTB, tag=f"qTz{b}{h}", name=f"qTz{b}{h}")
                for h in range(2)
            ])
            kTz.append([
                qk_pool.tile([128, S], DTB, tag=f"kTz{b}{h}", name=f"kTz{b}{h}")
                for h in range(2)
            ])
            v_sb.append(v_pool.tile([128, N_SB, 2, 128], DTB, tag=f"v{b}", name=f"v{b}"))

        def zero_pads(b):
            # rows 64:128 of the per-head q/k tiles stay zero (full-width
            # contraction keeps the PE activity monitor at K=8/8); v cols
            # 64 = ones (softmax denominator), 65:128 = zero
            for h in range(2):
                nc.vector.memset(qTz[b][h][64:128, :], 0.0)
                nc.vector.memset(kTz[b][h][64:128, :], 0.0)
            nc.vector.memset(
                v_sb[b].rearrange("p a h e -> p (a h) e")[:, :, 64:65], 1.0
            )
            nc.vector.memset(
                v_sb[b].rearrange("p a h e -> p (a h) e")[:, :, 65:128], 0.0
            )

        def load_xt_sc(b, sc, eng=None):
            eng = eng or nc.gpsimd
            for dc in range(N_DC):
                eng.dma_start(
                    out=xt_sb[b][:, dc, ds(sc * SC, SC)],
                    in_=xt_d[b, sc, dc],
                )

        def _qk_proj(b, sc, w_sb, bias_sb, dstz, scratch_tag):
            ps = psum.tile([128, SC], F32, tag="misc", bufs=2, name=f"ps_{scratch_tag}")
            for dc in range(N_DC):
                nc.tensor.matmul(
                    ps, w_sb[:, dc, :], xt_sb[b][:, dc, ds(sc * SC, SC)],
                    start=(dc == 0), stop=(dc == N_DC - 1),
                )
            nc.vector.tensor_scalar_add(
                out=dstz[0][0:64, ds(sc * SC, SC)], in0=ps[0:64, :],
                scalar1=bias_sb[0:64],
            )
            qs = qs_pool.tile([128, SC], DTB, tag=scratch_tag, name=f"qs_{scratch_tag}")
            nc.vector.tensor_scalar_add(
                out=qs[64:128, :], in0=ps[64:128, :], scalar1=bias_sb[64:128]
            )
            nc.sync.dma_start(
                out=dstz[1][0:64, ds(sc * SC, SC)], in_=qs[64:128, :]
            )

        def qkv_q(b, sc):
            _qk_proj(b, sc, wq_sb, bq_sb, qTz[b], "qsq")

        def qkv_k(b, sc):
            _qk_proj(b, sc, wk_sb, bk_sb, kTz[b], "qsk")

        def qkv_v(b, sc):
            psv = psum.tile([128, SC], F32, tag="misc", bufs=2)
            psv4 = psv.rearrange("p (j e) -> p j e", j=4)
            for j4 in range(4):
                for dc in range(N_DC):
                    nc.tensor.matmul(
                        psv4[:, j4, :],
                        xt_sb[b][:, dc, ds(sc * SC + j4 * 128, 128)],
                        wv_sb[:, dc, :],
                        start=(dc == 0), stop=(dc == N_DC - 1),
                    )
            pjhe = psv.rearrange("p (j h e) -> p j h e", j=4, h=2)
            bjhe = bv_bc.rearrange("p j (h e) -> p j h e", h=2)
            for h in range(2):
                nc.vector.tensor_add(
                    out=v_sb[b][:, ds(4 * sc, 4), h, 0:64],
                    in0=pjhe[:, :, h, :],
                    in1=bjhe[:, :, h, :],
                )

        def attn_qc(b, qc, pop_filler):
            ntb = 4 * qc + 4
            psy = [
                psum.tile([128, SC], F32, tag="psy", bufs=2, name=f"psy{b}_{qc}_{_}")
                for _ in range(2)
            ]
            for tb in range(ntb):
                psc = psum.tile([128, 2, SC], F32, tag="sc2", bufs=2)
                ex = exp_pool.tile([128, 2, SC], DTB)
                for h in range(2):
                    nc.tensor.matmul(
                        psc[:, h, :],
                        kTz[b][h][:, ds(tb * 128, 128)],
                        qTz[b][h][:, ds(qc * SC, SC)],
                        start=True, stop=True,
                    )
                pop_filler()
                nc.scalar.activation(
                    out=ex, in_=psc,
                    func=mybir.ActivationFunctionType.Exp,
                    scale=0.125,
                )
                j = tb - 4 * qc
                if j >= 0:
                    if j > 0:
                        nc.vector.memset(ex[:, :, 0 : j * 128], 0.0)
                    nc.vector.tensor_mul(
                        out=ex[:, :, ds(j * 128, 128)],
                        in0=ex[:, :, ds(j * 128, 128)],
                        in1=tri_sb,
                    )
                for h in range(2):
                    nc.tensor.matmul(
                        psy[h], v_sb[b][:, tb, h, :], ex[:, h, :],
                        start=(tb == 0), stop=(tb == ntb - 1),
                    )
            # normalize: one fast PSUM->SBUF copy per head releases psy;
            # the divide chain then runs entirely off the critical path
            for h in range(2):
                yhat = r_pool.tile([64, SC], DTB, name=f"yhat{h}")
                nc.vector.tensor_copy(out=yhat, in_=psy[h][0:64, :])
                zrow = r_pool.tile([1, SC], DTB, name=f"zrow{h}")
                nc.vector.tensor_copy(out=zrow, in_=psy[h][64:65, :])
                zb = psum.tile([128, SC], F32, tag="misc", bufs=2)
                nc.tensor.matmul(
                    zb[0:64, :], ones64_sb, zrow, start=True, stop=True
                )
                rbc = z_pool.tile([64, SC], F32)
                nc.vector.reciprocal_approx_fast(out=rbc, in_=zb[0:64, :])
                yts = yts_pool.tile([64, SC], DTB)
                nc.vector.tensor_mul(out=yts, in0=yhat, in1=rbc)
                nc.sync.dma_start(
                    out=y_part[b].ap()[ds(2 * qc, 2), ds(64 * h, 64), :].transpose(
                        [1, 0, 2]
                    ),
                    in_=yts.rearrange("p (c q) -> p c q", c=2),
                )

        ygs = [yg_pool.tile([128, N_CORES, CQ], DTB, tag=f"yg{b}", name=f"yg{b}") for b in range(B)]

        def load_ygs(b):
            nc.sync.dma_start(
                out=ygs[b], in_=y_all[b].ap().transpose([1, 0, 2])
            )

        def outproj_piece(b, qb, ch):
            pso = psum.tile([128, SC], F32, tag="misc", bufs=2)
            for ec in range(N_CORES):
                nc.tensor.matmul(
                    pso,
                    ygs[b][:, ec, ds(qb * 128, 128)],
                    wout_sb[:, ec, ds(ch * SC, SC)],
                    start=(ec == 0), stop=(ec == N_CORES - 1),
                )
            ot = outs_pool.tile([128, SC], F32)
            nc.vector.tensor_add(out=ot, in0=pso, in1=bout_bc[:, ds(ch * SC, SC)])
            nc.sync.dma_start(
                out=out_d[b, ds(qb * 128, 128), ds(ch * SC, SC)], in_=ot
            )

        def a2a(b):
            nc.gpsimd.collective_compute(
                "AllToAll",
                mybir.AluOpType.bypass,
                replica_groups=[list(range(N_CORES))],
                ins=[y_part[b].ap()],
                outs=[y_all[b].ap()],
            )

        # ================= emission =================
        # xt(b0,sc0) on the latency-critical sync queue; all other bulk
        # loads go through the (otherwise idle) gpsimd queue so small
        # attention-critical DMAs are never stuck behind them
        load_xt_sc(0, 0, eng=nc.sync)
        zero_pads(0)
        zero_pads(1)
        for sc in range(1, N_SC):
            load_xt_sc(0, sc)
        qkv_q(0, 0)
        qkv_k(0, 0)
        qkv_v(0, 0)
        for sc in range(N_SC):
            load_xt_sc(1, sc)
        nc.gpsimd.dma_start(
            out=wout_sb, in_=wout_d.rearrange("(c p) e -> p c e", p=128)
        )
        nc.gpsimd.dma_start(out=bout_bc, in_=bout_d.to_broadcast([128, D]))

        # remaining QKV work as an ordered unit queue; units with key
        # (b, sc) must be emitted before attn_qc(b, qc >= sc)
        units = []
        for key in [(0, 1), (0, 2), (0, 3), (1, 0), (1, 1), (1, 2), (1, 3)]:
            b, sc = key
            units.append((key, lambda b=b, sc=sc: qkv_q(b, sc)))
            units.append((key, lambda b=b, sc=sc: qkv_k(b, sc)))
            units.append((key, lambda b=b, sc=sc: qkv_v(b, sc)))

        def flush_to(key):
            while units and units[0][0] <= key:
                units.pop(0)[1]()

        def popper(limit_key):
            def pop():
                if units and units[0][0] <= limit_key:
                    units.pop(0)[1]()
            return pop

        for qc in range(N_SC):
            flush_to((0, qc))
            attn_qc(0, qc, popper((1, 0)))
        flush_to((1, 0))
        a2a(0)

        for qc in range(N_SC):
            flush_to((1, qc))
            attn_qc(1, qc, popper((1, 3)))
        a2a(1)

        load_ygs(0)
        for qb in range(2):
            for ch in range(2):
                outproj_piece(0, qb, ch)
        load_ygs(1)
        for qb in range(2):
            for ch in range(2):
                outproj_piece(1, qb, ch)

    nc.compile()
    return nc


_NC_CACHE = None


def _get_program():
    global _NC_CACHE
    if _NC_CACHE is None:
        _NC_CACHE = _build_program()
    return _NC_CACHE


def make_in_maps(x, Wqkv, bqkv, Wout, bout):
    x = np.asarray(x, dtype=np.float32)
    Wqkv = np.asarray(Wqkv, dtype=np.float32)
    bqkv = np.asarray(bqkv, dtype=np.float32)
    Wout = np.asarray(Wout, dtype=np.float32)
    bout = np.asarray(bout, dtype=np.float32)

    # [B, D, S] -> chunk-major [B, N_SC, N_DC, 128, SC] for contiguous DMA
    xt = (
        x.transpose(0, 2, 1)
        .reshape(B, N_DC, 128, N_SC, SC)
        .transpose(0, 3, 1, 2, 4)
    )
    xt = np.ascontiguousarray(xt).astype(NP_BF16)
    wout = np.ascontiguousarray(Wout).astype(NP_BF16)
    bout2 = np.ascontiguousarray(bout.reshape(1, D))

    in_maps = []
    for c in range(N_CORES):
        h0, h1 = 2 * c, 2 * c + 1
        wq = np.concatenate(
            [Wqkv[h0, :, 0:64], Wqkv[h1, :, 0:64]], axis=1
        ).astype(NP_BF16)
        wk = np.concatenate(
            [Wqkv[h0, :, 64:128], Wqkv[h1, :, 64:128]], axis=1
        ).astype(NP_BF16)
        wv = np.concatenate(
            [Wqkv[h0, :, 128:192], Wqkv[h1, :, 128:192]], axis=1
        ).astype(NP_BF16)
        bq = np.ascontiguousarray(
            np.concatenate([bqkv[h0, 0:64], bqkv[h1, 0:64]]).reshape(128, 1)
        )
        bk = np.ascontiguousarray(
            np.concatenate([bqkv[h0, 64:128], bqkv[h1, 64:128]]).reshape(128, 1)
        )
        bv = np.ascontiguousarray(
            np.concatenate([bqkv[h0, 128:192], bqkv[h1, 128:192]]).reshape(1, 128)
        )
        in_maps.append(
            {
                "xt": xt,
                "wq": np.ascontiguousarray(wq),
                "wk": np.ascontiguousarray(wk),
                "wv": np.ascontiguousarray(wv),
                "bq": bq,
                "bk": bk,
                "bv": bv,
                "wout": wout,
                "bout": bout2,
            }
        )
    return in_maps


def assemble(results):
    full = np.empty((B, S, D), dtype=np.float32)
    for c in range(N_CORES):
        full[:, 256 * c : 256 * (c + 1)] = results[c]["out"]
    return full


def _install_ntff_hook():
    """The agent image's antenv lacks axon_hooks; provide it so
    run_bass_kernel_spmd(trace=True) can NTFF-profile via libaxon."""
    if "antenv.axon_hooks" in sys.modules:
        return
    so_path = "/opt/axon/libaxon_pjrt.so"
    try:
        lib = ctypes.CDLL(so_path)
        lib.axon_start_nrt_profile.argtypes = [
            ctypes.POINTER(ctypes.c_int64),
            ctypes.c_size_t,
        ]
        lib.axon_start_nrt_profile.restype = ctypes.c_int64
        lib.axon_stop_nrt_profile.argtypes = [ctypes.c_char_p]
        lib.axon_stop_nrt_profile.restype = ctypes.c_int64
    except (OSError, AttributeError):
        return

    @contextlib.contextmanager
    def _hook(output_dir, device_ids):
        import jax

        jax.devices()
        if device_ids:
            ids = (ctypes.c_int64 * len(device_ids))(*device_ids)
            rc = lib.axon_start_nrt_profile(ids, len(device_ids))
        else:
            rc = lib.axon_start_nrt_profile(None, 0)
        if rc != 0:
            raise RuntimeError(f"axon_start_nrt_profile rc={rc}")
        try:
            yield
        finally:
            n = lib.axon_stop_nrt_profile(str(output_dir).encode())
            if n < 0:
                raise RuntimeError(f"axon_stop_nrt_profile rc={n}")

    mod = types.ModuleType("antenv.axon_hooks")
    mod.get_axon_ntff_profile_hook = lambda: _hook
    mod.set_axon_ntff_profile_hook = lambda h: None
    sys.modules["antenv.axon_hooks"] = mod


def run(inputs, trace=False):
    """Run on the 8 NeuronCores. Returns (output, BassKernelResults)."""
    from concourse.bass_utils import run_bass_kernel_spmd

    if trace:
        _install_ntff_hook()
    nc = _get_program()
    in_maps = make_in_maps(**inputs)
    res = run_bass_kernel_spmd(
        nc, in_maps, core_ids=list(range(N_CORES)), trace=trace
    )
    return assemble(res.results), res


def kernel(x, Wqkv, bqkv, Wout, bout):
    out, _ = run(
        {"x": x, "Wqkv": Wqkv, "bqkv": bqkv, "Wout": Wout, "bout": bout},
        trace=False,
    )
    return out


# revision 15
# speedup vs baseline: 1.7657x; 1.0502x over previous
"""Trainium2 Bass kernel for naive causal MHA (dense transformer block).

Problem: x[2, 2048, 1024], per-head QKV (16 heads, head_dim 64), causal
softmax attention, concat heads, output projection.

Sharding (8 NeuronCores, tensor-parallel over heads):
  - core c computes QKV + attention for heads {2c, 2c+1} over both batches
    in a transposed layout: scores are built as [keys, queries] so the
    softmax denominator comes from an extra ones-column in V and the
    attention output lands directly in the [head_dim, seq] layout the
    output projection needs as its stationary operand.
  - one 8-way AllToAll PER BATCH reshards y from head-split to row-split
    (the batch-0 collective and output projection overlap batch-1 compute),
  - each core computes a disjoint 256-row slice of y @ Wout + bout per batch.

Perf notes vs the f32r baseline (450 us):
  - all matmuls in bf16 (f32r moving operands stream at half rate),
  - exp over [128, 4*512] groups spanning 4 PSUM banks (2 t-blocks x 2
    heads) to amortize the ~352-cycle ACTIVATE instruction overhead,
  - softmax normalization via reciprocal_approx_fast + a PE ones-broadcast
    matmul instead of a DVE iterative reciprocal + DRAM round-trip,
  - QKV(b1) / out-proj(b0) matmuls are interleaved into the ACT-bound
    attention instruction stream to fill PE bubbles.
"""

import contextlib
import ctypes
import sys
import types

import numpy as np

import concourse.bacc as bacc
import concourse.mybir as mybir
import concourse.tile as tile
from concourse.bass import ds

N_CORES = 8
B = 2
S = 2048
D = 1024
HD = 64
N_HEADS = 16

F32 = mybir.dt.float32
DTB = mybir.dt.bfloat16
NP_BF16 = mybir.dt.np(mybir.dt.bfloat16)

SC = 512          # seq chunk (moving-operand width)
N_SC = S // SC    # 4
N_DC = D // 128   # 8 contraction chunks
N_SB = S // 128   # 16 seq 128-blocks
CQ = S // N_CORES // B  # 128... no: per-batch a2a slot width = 2048/8 = 256
CQ = S // N_CORES       # 256 q per a2a slot


def _f32r(ap):
    return ap.bitcast(mybir.dt.float32r)


def _mask_np():
    """Upper-triangular keep-mask (t <= q) for the diagonal 128x128 score
    block, duplicated for both heads: [t, h, q]."""
    tri = np.triu(np.ones((128, 128), dtype=np.float32))
    return np.stack([tri, tri], axis=1).astype(NP_BF16)


def _build_program():
    nc = bacc.Bacc(
        "TRN2", target_bir_lowering=False, debug=False, num_devices=N_CORES
    )

    xt_d = nc.dram_tensor("xt", [B, N_SC, N_DC, 128, SC], DTB, kind="ExternalInput").ap()
    wq_d = nc.dram_tensor("wq", [D, 128], DTB, kind="ExternalInput").ap()
    wk_d = nc.dram_tensor("wk", [D, 128], DTB, kind="ExternalInput").ap()
    wv_d = nc.dram_tensor("wv", [D, 128], DTB, kind="ExternalInput").ap()
    bq_d = nc.dram_tensor("bq", [128, 1], F32, kind="ExternalInput").ap()
    bk_d = nc.dram_tensor("bk", [128, 1], F32, kind="ExternalInput").ap()
    bv_d = nc.dram_tensor("bv", [1, 128], F32, kind="ExternalInput").ap()
    wout_d = nc.dram_tensor("wout", [D, D], DTB, kind="ExternalInput").ap()
    bout_d = nc.dram_tensor("bout", [1, D], F32, kind="ExternalInput").ap()
    out_d = nc.dram_tensor("out", [B, 2 * 128, D], F32, kind="ExternalOutput").ap()

    y_part = [nc.dram_tensor(f"y{b}p", [N_CORES, 128, CQ], DTB) for b in range(B)]
    y_all = [nc.dram_tensor(f"y{b}a", [N_CORES, 128, CQ], DTB) for b in range(B)]

    mask_d = nc.inline_tensor(_mask_np(), name="tri")
    ones64_d = nc.inline_tensor(np.ones((1, 64), dtype=NP_BF16), name="ones64")

    with tile.TileContext(nc) as tc, contextlib.ExitStack() as ctx:
        const = ctx.enter_context(tc.tile_pool(name="const", bufs=1))
        xt_pool = ctx.enter_context(tc.tile_pool(name="xt", bufs=1))
        qk_pool = ctx.enter_context(tc.tile_pool(name="qk", bufs=1))
        v_pool = ctx.enter_context(tc.tile_pool(name="vp", bufs=1))
        exp_pool = ctx.enter_context(tc.tile_pool(name="expp", bufs=2))
        r_pool = ctx.enter_context(tc.tile_pool(name="rp", bufs=2))
        z_pool = ctx.enter_context(tc.tile_pool(name="zp", bufs=2))
        yts_pool = ctx.enter_context(tc.tile_pool(name="yts", bufs=3))
        qs_pool = ctx.enter_context(tc.tile_pool(name="qs", bufs=2))
        yg_pool = ctx.enter_context(tc.tile_pool(name="yg", bufs=1))
        outs_pool = ctx.enter_context(tc.tile_pool(name="outs", bufs=2))
        psum = ctx.enter_context(tc.tile_pool(name="psum", bufs=1, space="PSUM"))

        # ---- constants into SBUF (critical-path order: wq + biases
        # before the bulk; slow partition-broadcast loads on gpsimd) ----
        wq_sb = const.tile([128, N_DC, 128], DTB)
        nc.sync.dma_start(out=wq_sb, in_=wq_d.rearrange("(c p) e -> p c e", p=128))
        bq_sb = const.tile([128, 1], F32)
        nc.sync.dma_start(out=bq_sb, in_=bq_d)
        bk_sb = const.tile([128, 1], F32)
        nc.sync.dma_start(out=bk_sb, in_=bk_d)
        wk_sb = const.tile([128, N_DC, 128], DTB)
        nc.sync.dma_start(out=wk_sb, in_=wk_d.rearrange("(c p) e -> p c e", p=128))
        wv_sb = const.tile([128, N_DC, 128], DTB)
        nc.sync.dma_start(out=wv_sb, in_=wv_d.rearrange("(c p) e -> p c e", p=128))
        tri_sb = const.tile([128, 2, 128], DTB)
        nc.sync.dma_start(out=tri_sb, in_=mask_d.ap())
        ones64_sb = const.tile([1, 64], DTB)
        nc.sync.dma_start(out=ones64_sb, in_=ones64_d.ap())
        wout_sb = const.tile([128, N_DC, D], DTB)
        bout_bc = const.tile([128, D], F32)
        bv_bc = const.tile([128, 4, 128], F32)
        for j in range(4):
            nc.gpsimd.dma_start(out=bv_bc[:, j, :], in_=bv_d.to_broadcast([128, 128]))

        # ---- per-batch persistent SBUF ----
        xt_sb = []
        qTz, kTz, v_sb = [], [], []
        for b in range(B):
            xt_sb.append(xt_pool.tile([128, N_DC, S], DTB, tag=f"xt{b}", name=f"xt{b}"))
            qTz.append([
                qk_pool.tile([128, S], DTB, tag=f"qTz{b}{h}", name=f"qTz{b}{h}")
                for h in range(2)
            ])
            kTz.append([
                qk_pool.tile([128, S], DTB, tag=f"kTz{b}{h}", name=f"kTz{b}{h}")
                for h in range(2)
            ])
            v_sb.append(v_pool.tile([128, N_SB, 2, 128], DTB, tag=f"v{b}", name=f"v{b}"))

        def zero_pads(b):
            # whole-tile zero via int32 bitcast (fast 4x memset); QKV fills
            # rows/cols 0:64 later. Rows 64:128 of q/k stay zero so the
            # full-width contraction keeps the PE activity monitor happy;
            # v col 64 = ones (softmax denominator), 65:128 stay zero.
            for h in range(2):
                nc.vector.memset(qTz[b][h].bitcast(mybir.dt.int32), 0)
                nc.vector.memset(kTz[b][h].bitcast(mybir.dt.int32), 0)
            nc.vector.memset(
                v_sb[b].rearrange("p a h e -> p (a h e)").bitcast(mybir.dt.int32), 0
            )
            nc.vector.memset(
                v_sb[b].rearrange("p a h e -> p (a h) e")[:, :, 64:65], 1.0
            )

        def load_xt_sc(b, sc, eng=None):
            eng = eng or nc.gpsimd
            for dc in range(N_DC):
                eng.dma_start(
                    out=xt_sb[b][:, dc, ds(sc * SC, SC)],
                    in_=xt_d[b, sc, dc],
                )

        def _qk_proj(b, sc, w_sb, bias_sb, dstz, scratch_tag):
            ps = psum.tile([128, SC], F32, tag="misc", bufs=2, name=f"ps_{scratch_tag}")
            for dc in range(N_DC):
                nc.tensor.matmul(
                    ps, w_sb[:, dc, :], xt_sb[b][:, dc, ds(sc * SC, SC)],
                    start=(dc == 0), stop=(dc == N_DC - 1),
                )
            nc.vector.tensor_scalar_add(
                out=dstz[0][0:64, ds(sc * SC, SC)], in0=ps[0:64, :],
                scalar1=bias_sb[0:64],
            )
            qs = qs_pool.tile([128, SC], DTB, tag=scratch_tag, name=f"qs_{scratch_tag}")
            nc.vector.tensor_scalar_add(
                out=qs[64:128, :], in0=ps[64:128, :], scalar1=bias_sb[64:128]
            )
            nc.sync.dma_start(
                out=dstz[1][0:64, ds(sc * SC, SC)], in_=qs[64:128, :]
            )

        def qkv_q(b, sc):
            _qk_proj(b, sc, wq_sb, bq_sb, qTz[b], "qsq")

        def qkv_k(b, sc):
            _qk_proj(b, sc, wk_sb, bk_sb, kTz[b], "qsk")

        def qkv_v(b, sc):
            psv = psum.tile([128, SC], F32, tag="misc", bufs=2)
            psv4 = psv.rearrange("p (j e) -> p j e", j=4)
            for j4 in range(4):
                for dc in range(N_DC):
                    nc.tensor.matmul(
                        psv4[:, j4, :],
                        xt_sb[b][:, dc, ds(sc * SC + j4 * 128, 128)],
                        wv_sb[:, dc, :],
                        start=(dc == 0), stop=(dc == N_DC - 1),
                    )
            pjhe = psv.rearrange("p (j h e) -> p j h e", j=4, h=2)
            bjhe = bv_bc.rearrange("p j (h e) -> p j h e", h=2)
            for h in range(2):
                nc.vector.tensor_add(
                    out=v_sb[b][:, ds(4 * sc, 4), h, 0:64],
                    in0=pjhe[:, :, h, :],
                    in1=bjhe[:, :, h, :],
                )

        def attn_qc(b, qc, pop_filler):
            ntb = 4 * qc + 4
            psy = [
                psum.tile([128, SC], F32, tag="psy", bufs=2, name=f"psy{b}_{qc}_{_}")
                for _ in range(2)
            ]
            for tb in range(ntb):
                j = tb - 4 * qc
                # diagonal blocks only need columns >= j*128 (t <= q)
                o = j * 128 if j > 0 else 0
                w = SC - o
                psc = psum.tile([128, 2, SC], F32, tag="sc2", bufs=2)
                ex = exp_pool.tile([128, 2, SC], DTB)
                for h in range(2):
                    nc.tensor.matmul(
                        psc[:, h, ds(o, w)],
                        kTz[b][h][:, ds(tb * 128, 128)],
                        qTz[b][h][:, ds(qc * SC + o, w)],
                        start=True, stop=True,
                    )
                pop_filler()
                nc.scalar.activation(
                    out=ex[:, :, ds(o, w)], in_=psc[:, :, ds(o, w)],
                    func=mybir.ActivationFunctionType.Exp,
                    scale=0.125,
                )
                if j >= 0:
                    nc.vector.tensor_mul(
                        out=ex[:, :, ds(j * 128, 128)],
                        in0=ex[:, :, ds(j * 128, 128)],
                        in1=tri_sb,
                    )
                for h in range(2):
                    nc.tensor.matmul(
                        psy[h][:, ds(o, w)], v_sb[b][:, tb, h, :],
                        ex[:, h, ds(o, w)],
                        start=(tb == 0), stop=(tb == ntb - 1),
                        skip_group_check=True,
                    )
            # normalize: one fast PSUM->SBUF copy per head releases psy;
            # the divide chain then runs entirely off the critical path
            for h in range(2):
                yhat = r_pool.tile([64, SC], DTB, name=f"yhat{h}")
                nc.vector.tensor_copy(out=yhat, in_=psy[h][0:64, :])
                zrow = r_pool.tile([1, SC], DTB, name=f"zrow{h}")
                nc.vector.tensor_copy(out=zrow, in_=psy[h][64:65, :])
                zb = psum.tile([128, SC], F32, tag="misc", bufs=2)
                nc.tensor.matmul(
                    zb[0:64, :], ones64_sb, zrow, start=True, stop=True
                )
                rbc = z_pool.tile([64, SC], F32)
                nc.vector.reciprocal_approx_fast(out=rbc, in_=zb[0:64, :])
                yts = yts_pool.tile([64, SC], DTB)
                nc.vector.tensor_mul(out=yts, in0=yhat, in1=rbc)
                nc.sync.dma_start(
                    out=y_part[b].ap()[ds(2 * qc, 2), ds(64 * h, 64), :].transpose(
                        [1, 0, 2]
                    ),
                    in_=yts.rearrange("p (c q) -> p c q", c=2),
                )

        ygs = [yg_pool.tile([128, N_CORES, CQ], DTB, tag=f"yg{b}", name=f"yg{b}") for b in range(B)]

        def load_ygs(b):
            nc.sync.dma_start(
                out=ygs[b], in_=y_all[b].ap().transpose([1, 0, 2])
            )

        def outproj_piece(b, qb, ch):
            pso = psum.tile([128, SC], F32, tag="misc", bufs=2)
            for ec in range(N_CORES):
                nc.tensor.matmul(
                    pso,
                    ygs[b][:, ec, ds(qb * 128, 128)],
                    wout_sb[:, ec, ds(ch * SC, SC)],
                    start=(ec == 0), stop=(ec == N_CORES - 1),
                )
            ot = outs_pool.tile([128, SC], F32)
            nc.vector.tensor_add(out=ot, in0=pso, in1=bout_bc[:, ds(ch * SC, SC)])
            nc.sync.dma_start(
                out=out_d[b, ds(qb * 128, 128), ds(ch * SC, SC)], in_=ot
            )

        def a2a(b):
            nc.gpsimd.collective_compute(
                "AllToAll",
                mybir.AluOpType.bypass,
                replica_groups=[list(range(N_CORES))],
                ins=[y_part[b].ap()],
                outs=[y_all[b].ap()],
            )

        # ================= emission =================
        # xt(b0,sc0) on the latency-critical sync queue; all other bulk
        # loads go through the (otherwise idle) gpsimd queue so small
        # attention-critical DMAs are never stuck behind them
        load_xt_sc(0, 0, eng=nc.sync)
        zero_pads(0)
        for sc in range(1, N_SC):
            load_xt_sc(0, sc)
        qkv_q(0, 0)
        qkv_k(0, 0)
        qkv_v(0, 0)
        zero_pads(1)
        for sc in range(N_SC):
            load_xt_sc(1, sc)
        nc.gpsimd.dma_start(
            out=wout_sb, in_=wout_d.rearrange("(c p) e -> p c e", p=128)
        )
        nc.gpsimd.dma_start(out=bout_bc, in_=bout_d.to_broadcast([128, D]))

        # remaining QKV work as an ordered unit queue; units with key
        # (b, sc) must be emitted before attn_qc(b, qc >= sc)
        units = []
        for key in [(0, 1), (0, 2), (0, 3), (1, 0), (1, 1), (1, 2), (1, 3)]:
            b, sc = key
            units.append((key, lambda b=b, sc=sc: qkv_q(b, sc)))
            units.append((key, lambda b=b, sc=sc: qkv_k(b, sc)))
            units.append((key, lambda b=b, sc=sc: qkv_v(b, sc)))

        def flush_to(key):
            while units and units[0][0] <= key:
                units.pop(0)[1]()

        def popper(limit_key):
            def pop():
                if units and units[0][0] <= limit_key:
                    units.pop(0)[1]()
            return pop

        for qc in range(N_SC):
            flush_to((0, qc))
            attn_qc(0, qc, popper((1, 0)))
        flush_to((1, 0))
        a2a(0)

        for qc in range(N_SC):
            flush_to((1, qc))
            attn_qc(1, qc, popper((1, 3)))
        a2a(1)

        load_ygs(0)
        for qb in range(2):
            for ch in range(2):
                outproj_piece(0, qb, ch)
        load_ygs(1)
        for qb in range(2):
            for ch in range(2):
                outproj_piece(1, qb, ch)

    nc.compile()
    return nc


_NC_CACHE = None


def _get_program():
    global _NC_CACHE
    if _NC_CACHE is None:
        _NC_CACHE = _build_program()
    return _NC_CACHE


def make_in_maps(x, Wqkv, bqkv, Wout, bout):
    x = np.asarray(x, dtype=np.float32)
    Wqkv = np.asarray(Wqkv, dtype=np.float32)
    bqkv = np.asarray(bqkv, dtype=np.float32)
    Wout = np.asarray(Wout, dtype=np.float32)
    bout = np.asarray(bout, dtype=np.float32)

    # [B, D, S] -> chunk-major [B, N_SC, N_DC, 128, SC] for contiguous DMA
    xt = (
        x.transpose(0, 2, 1)
        .reshape(B, N_DC, 128, N_SC, SC)
        .transpose(0, 3, 1, 2, 4)
    )
    xt = np.ascontiguousarray(xt).astype(NP_BF16)
    wout = np.ascontiguousarray(Wout).astype(NP_BF16)
    bout2 = np.ascontiguousarray(bout.reshape(1, D))

    in_maps = []
    for c in range(N_CORES):
        h0, h1 = 2 * c, 2 * c + 1
        wq = np.concatenate(
            [Wqkv[h0, :, 0:64], Wqkv[h1, :, 0:64]], axis=1
        ).astype(NP_BF16)
        wk = np.concatenate(
            [Wqkv[h0, :, 64:128], Wqkv[h1, :, 64:128]], axis=1
        ).astype(NP_BF16)
        wv = np.concatenate(
            [Wqkv[h0, :, 128:192], Wqkv[h1, :, 128:192]], axis=1
        ).astype(NP_BF16)
        bq = np.ascontiguousarray(
            np.concatenate([bqkv[h0, 0:64], bqkv[h1, 0:64]]).reshape(128, 1)
        )
        bk = np.ascontiguousarray(
            np.concatenate([bqkv[h0, 64:128], bqkv[h1, 64:128]]).reshape(128, 1)
        )
        bv = np.ascontiguousarray(
            np.concatenate([bqkv[h0, 128:192], bqkv[h1, 128:192]]).reshape(1, 128)
        )
        in_maps.append(
            {
                "xt": xt,
                "wq": np.ascontiguousarray(wq),
                "wk": np.ascontiguousarray(wk),
                "wv": np.ascontiguousarray(wv),
                "bq": bq,
                "bk": bk,
                "bv": bv,
                "wout": wout,
                "bout": bout2,
            }
        )
    return in_maps


def assemble(results):
    full = np.empty((B, S, D), dtype=np.float32)
    for c in range(N_CORES):
        full[:, 256 * c : 256 * (c + 1)] = results[c]["out"]
    return full


def _install_ntff_hook():
    """The agent image's antenv lacks axon_hooks; provide it so
    run_bass_kernel_spmd(trace=True) can NTFF-profile via libaxon."""
    if "antenv.axon_hooks" in sys.modules:
        return
    so_path = "/opt/axon/libaxon_pjrt.so"
    try:
        lib = ctypes.CDLL(so_path)
        lib.axon_start_nrt_profile.argtypes = [
            ctypes.POINTER(ctypes.c_int64),
            ctypes.c_size_t,
        ]
        lib.axon_start_nrt_profile.restype = ctypes.c_int64
        lib.axon_stop_nrt_profile.argtypes = [ctypes.c_char_p]
        lib.axon_stop_nrt_profile.restype = ctypes.c_int64
    except (OSError, AttributeError):
        return

    @contextlib.contextmanager
    def _hook(output_dir, device_ids):
        import jax

        jax.devices()
        if device_ids:
            ids = (ctypes.c_int64 * len(device_ids))(*device_ids)
            rc = lib.axon_start_nrt_profile(ids, len(device_ids))
        else:
            rc = lib.axon_start_nrt_profile(None, 0)
        if rc != 0:
            raise RuntimeError(f"axon_start_nrt_profile rc={rc}")
        try:
            yield
        finally:
            n = lib.axon_stop_nrt_profile(str(output_dir).encode())
            if n < 0:
                raise RuntimeError(f"axon_stop_nrt_profile rc={n}")

    mod = types.ModuleType("antenv.axon_hooks")
    mod.get_axon_ntff_profile_hook = lambda: _hook
    mod.set_axon_ntff_profile_hook = lambda h: None
    sys.modules["antenv.axon_hooks"] = mod


def run(inputs, trace=False):
    """Run on the 8 NeuronCores. Returns (output, BassKernelResults)."""
    from concourse.bass_utils import run_bass_kernel_spmd

    if trace:
        _install_ntff_hook()
    nc = _get_program()
    in_maps = make_in_maps(**inputs)
    res = run_bass_kernel_spmd(
        nc, in_maps, core_ids=list(range(N_CORES)), trace=trace
    )
    return assemble(res.results), res


def kernel(x, Wqkv, bqkv, Wout, bout):
    out, _ = run(
        {"x": x, "Wqkv": Wqkv, "bqkv": bqkv, "Wout": Wout, "bout": bout},
        trace=False,
    )
    return out
